# revision 11
# baseline (speedup 1.0000x reference)
"""Trainium2 Bass kernel for nn_CGCNNDynamics (Point-BERT DGCNN dynamics head).

V2: pair-split (2 cores per batch element, channel halves at GN group
boundaries) + instruction-count-oriented restructure:
 - one combined [hT | sT] matmul pass per point chunk
 - one 4-neighbor indirect gather per query chunk (multi-index offset AP)
 - edge values e = g + s formed explicitly; GN stats = sum(e), sum(e^2)
   accumulated across the whole block in two [1, OH] PSUM rows via
   ones-stationary matmuls (2 per query chunk)
 - channel-major staging + XBAR dma_start_transpose (per-slice 128x128)
   replaces per-chunk PE transposes; one wide Prelu per output-channel chunk
 - merged DMAs (4-chunk htbl writes, 1 DMA per block for agin/fnext)
 - pair AllGather of bf16 channel halves between blocks
"""
import sys, os
sys.path.insert(0, "/opt/trn_rl_repo")
KSTOP = os.environ.get("KSTOP", "")
import contextlib
import numpy as np
import ml_dtypes

import concourse.bass as bass
import concourse.bacc as bacc
import concourse.mybir as mybir
import concourse.tile as tile
from concourse.tile import add_dep_helper
from concourse.bass_utils import run_bass_kernel_spmd

BF16 = ml_dtypes.bfloat16
P = 128
B, N, TD, AD, DD = 4, 2048, 256, 8, 512
DDH = DD // 2
CIN = TD + AD          # 264
K = 4
EPS = 1e-5
ALPHA = 0.2
NCH = N // P           # 16
BLOCKS = [(128, 128), (256, 256), (512, 256), (512, 512)]   # (C_full, O_half)
PAIRS = [[0, 1], [2, 3], [4, 5], [6, 7]]

_cache = {}


def _build_nc():
    nc = bacc.Bacc("TRN2", target_bir_lowering=False, debug=False,
                   enable_asserts=False, num_devices=8)
    f32, bf16, u32 = mybir.dt.float32, mybir.dt.bfloat16, mybir.dt.uint32

    xaug = nc.dram_tensor("xaug", [CIN + 1, N], bf16, kind="ExternalInput").ap()
    wina = nc.dram_tensor("wina", [CIN + 1, P], bf16, kind="ExternalInput").ap()
    knnl = nc.dram_tensor("knnl", [4, N // 2], f32, kind="ExternalInput").ap()
    knnr = nc.dram_tensor("knnr", [4, N], f32, kind="ExternalInput").ap()
    wstk = [nc.dram_tensor(f"wstk{i+1}", [c, 2 * o], bf16, kind="ExternalInput").ap()
            for i, (c, o) in enumerate(BLOCKS)]
    gnp = [nc.dram_tensor(f"gnp{i+1}", [1, 2048], f32, kind="ExternalInput").ap()
           for i in range(4)]
    w5a = nc.dram_tensor("w5a", [19 * P, DDH], bf16, kind="ExternalInput").ap()
    gn5t = nc.dram_tensor("gn5t", [DDH, 2], f32, kind="ExternalInput").ap()
    outT = nc.dram_tensor("outT", [DDH, N], f32, kind="ExternalOutput").ap()

    htbl = [nc.dram_tensor(f"htbl{i+1}", [N, o], bf16, kind="Internal").ap()
            for i, (_, o) in enumerate(BLOCKS)]
    agin = [nc.dram_tensor(f"agin{i+1}", [o, N], bf16, kind="Internal").ap()
            for i, (_, o) in enumerate(BLOCKS)]
    agout = [nc.dram_tensor(f"agout{i+1}", [2 * o, N], bf16, kind="Internal").ap()
             for i, (_, o) in enumerate(BLOCKS)]
    idxin = nc.dram_tensor("idxin", [P, 64], mybir.dt.uint32, kind="Internal").ap()
    idxout = nc.dram_tensor("idxout", [2 * P, 64], mybir.dt.uint32,
                            kind="Internal").ap()

    with tile.TileContext(nc) as tc:
        _emit(nc, tc, xaug, wina, knnl, knnr, wstk, gnp, w5a, gn5t,
              outT, htbl, agin, agout, idxin, idxout)
    nc.compile()
    return nc


def _emit(nc, tc, xaug, wina, knnl, knnr, wstk, gnp, w5a, gn5t,
          outT, htbl, agin, agout, idxin, idxout):
    f32, bf16, u32 = mybir.dt.float32, mybir.dt.bfloat16, mybir.dt.uint32
    AX, ALU, AF = mybir.AxisListType, mybir.AluOpType, mybir.ActivationFunctionType
    from concourse.masks import make_identity

    ctx = contextlib.ExitStack()
    with ctx:
        fpool = ctx.enter_context(tc.tile_pool(name="fpool", bufs=2))
        wpool = ctx.enter_context(tc.tile_pool(name="wpool", bufs=1))
        gpool = ctx.enter_context(tc.tile_pool(name="gpool", bufs=3))
        ftrp = ctx.enter_context(tc.tile_pool(name="ftrp", bufs=2))
        epool = ctx.enter_context(tc.tile_pool(name="epool", bufs=3))
        fmaxp = ctx.enter_context(tc.tile_pool(name="fmaxp", bufs=1))
        stallp = ctx.enter_context(tc.tile_pool(name="stallp", bufs=1))
        dstwp = ctx.enter_context(tc.tile_pool(name="dstwp", bufs=1))
        hstp = ctx.enter_context(tc.tile_pool(name="hstp", bufs=1))
        st1 = ctx.enter_context(tc.tile_pool(name="st1", bufs=1))
        st3 = ctx.enter_context(tc.tile_pool(name="st3", bufs=2))
        smalls = ctx.enter_context(tc.tile_pool(name="smalls", bufs=1))
        ps_mm = ctx.enter_context(tc.tile_pool(name="ps_mm", bufs=4, space="PSUM"))
        ps_st = ctx.enter_context(tc.tile_pool(name="ps_st", bufs=1, space="PSUM"))
        ps_tr = ctx.enter_context(tc.tile_pool(name="ps_tr", bufs=1, space="PSUM"))

        # ---- constants ----
        idt = smalls.tile([P, P], f32, tag="idt")
        make_identity(nc, idt[:])
        ones_col = smalls.tile([P, 1], f32, tag="ones_col")
        nc.vector.memset(ones_col[:], 1.0)
        ones_colb = smalls.tile([P, 1], bf16, tag="ones_colb")
        nc.vector.memset(ones_colb[:], 1.0)
        ones_row = smalls.tile([1, P], f32, tag="ones_row")
        nc.vector.memset(ones_row[:], 1.0)
        ones_rhs = smalls.tile([P, 512], bf16, tag="ones_rhs")
        nc.vector.memset(ones_rhs[:], 1.0)
        alpha_col = smalls.tile([P, 1], f32, tag="alpha_col")
        nc.vector.memset(alpha_col[:], ALPHA)
        eps_b = smalls.tile([1, 1], f32, tag="eps_b")
        nc.vector.memset(eps_b[:], EPS)
        eps_col = smalls.tile([P, 1], f32, tag="eps_col")
        nc.vector.memset(eps_col[:], EPS)
        idx_all = smalls.tile([P, NCH, 8], u32, tag="idx")
        top8v = smalls.tile([P, 8], f32, tag="top8v")
        wia = smalls.tile([P, 3, P], bf16, tag="wia")
        g5 = smalls.tile([P, 2, 2], f32, tag="g5")
        acc = smalls.tile([P, 2, 8], f32, tag="acc")
        acc2 = smalls.tile([P, 2, 8], f32, tag="acc2")

        def _bail():
            z = st1.tile([P, 512], f32, tag="sq5", name="bailz")
            nc.vector.memset(z[:], 0.0)
            for o5_ in range(DDH // P):
                for qs_ in range(N // 512):
                    nc.sync.dma_start(
                        outT[P * o5_:P * (o5_ + 1), 512 * qs_:512 * (qs_ + 1)], z[:])
        # ============ Phase A: conv_in -> f0 (chan-part bf16) ============
        xg = wpool.tile([P, 3, N], bf16, tag="w")
        xgv = xg[:]
        nc.sync.dma_start(xgv[:, 0, :], xaug[0:P, :])
        nc.sync.dma_start(xgv[:, 1, :], xaug[P:2 * P, :])
        nc.sync.dma_start(xgv[0:9, 2, :], xaug[2 * P:CIN + 1, :])
        nc.sync.dma_start(wia[:, 0, :], wina[0:P, :])
        nc.sync.dma_start(wia[:, 1, :], wina[P:2 * P, :])
        nc.sync.dma_start(wia[0:9, 2, :], wina[2 * P:CIN + 1, :])

        f0 = fpool.tile([P, 4, N], bf16, tag="f", name="f0")
        for qs in range(N // 512):
            pt = ps_mm.tile([P, 512], f32, tag="mm")
            sl = slice(512 * qs, 512 * (qs + 1))
            nc.tensor.matmul(out=pt[:], lhsT=wia[:, 0, :], rhs=xgv[:, 0, sl],
                             start=True, stop=False)
            nc.tensor.matmul(out=pt[:], lhsT=wia[:, 1, :], rhs=xgv[:, 1, sl],
                             start=False, stop=False)
            nc.tensor.matmul(out=pt[:], lhsT=wia[0:9, 2, :], rhs=xgv[0:9, 2, sl],
                             start=False, stop=True)
            nc.scalar.copy(f0[:, 0, sl], pt[:])

        if KSTOP == "a":
            _bail()
            return
        # ============ Phase B: KNN top-4 indices ============
        kl = smalls.tile([4, N // 2], f32, tag="kl")
        nc.sync.dma_start(kl[:], knnl)
        kr = smalls.tile([4, N], f32, tag="kr")
        nc.sync.dma_start(kr[:], knnr)
        for qc in range(NCH // 2):
            dsb = fmaxp.tile([P, 4, NCH, P], bf16, tag="fmax", name=f"dv{qc}")
            dview = dsb[:].rearrange("p a b c -> p (a b c)").bitcast(f32)[:, 0:2048]
            for js in range(N // 512):
                pt = ps_mm.tile([P, 512], f32, tag="mm")
                nc.tensor.matmul(out=pt[:], lhsT=kl[:, P * qc:P * (qc + 1)],
                                 rhs=kr[:, 512 * js:512 * (js + 1)],
                                 start=True, stop=True)
                nc.scalar.copy(dview[:, 512 * js:512 * (js + 1)], pt[:])
            nc.vector.max(out=top8v[:], in_=dview)
            nc.vector.max_index(out=idx_all[:, qc, :], in_max=top8v[:],
                                in_values=dview)
        # exchange halves: own idx -> slot h of idxout
        iw = nc.sync.dma_start(idxin, idx_all[:, 0:NCH // 2, :].rearrange(
            "p c o -> p (c o)"))
        if os.environ.get("NOCC", ""):
            icc = nc.gpsimd.dma_start(idxout[0:P, :], idxin)
            ic2 = nc.gpsimd.dma_start(idxout[P:2 * P, :], idxin)
            add_dep_helper(ic2.ins, iw.ins, reason="idx ag after idxin")
        else:
            icc = nc.gpsimd.collective_compute(
                "AllGather", ALU.bypass, replica_groups=PAIRS,
                ins=[idxin], outs=[idxout])
        add_dep_helper(icc.ins, iw.ins, reason="idx ag after idxin")
        il1 = nc.sync.dma_start(idx_all[:, 0:NCH // 2, :].rearrange(
            "p c o -> p (c o)"), idxout[0:P, :])
        il2 = nc.sync.dma_start(idx_all[:, NCH // 2:NCH, :].rearrange(
            "p c o -> p (c o)"), idxout[P:2 * P, :])
        add_dep_helper(il1.ins, icc.ins, reason="idx load after ag")
        add_dep_helper(il2.ins, icc.ins, reason="idx load after ag")

        if KSTOP == "b":
            _bail()
            return
        # ============ Edge blocks ============
        fprev = f0
        prevCC = 1
        ag_cc = [None] * 4
        for bi, (C, OH) in enumerate(BLOCKS):
            CC = C // P
            OC = OH // P
            grp = OH // 2
            # one DMA for the whole weight stack [C, 2*OH] -> [P, CC, 2*OH]
            wk = wpool.tile([P, 8, 512], bf16, tag="w")
            wv = wk[:].rearrange("p c o -> p (c o)")[:, 0:CC * 2 * OH].rearrange(
                "p (c o) -> p c o", o=2 * OH)
            nc.scalar.dma_start(
                wv[:], wstk[bi].rearrange("(c p) o -> p c o", p=P))

            # ---- C.1: combined [hT | sT] pass ----
            # st_all: sT stash bf16 [P, NCH, OH]; htbl written 4 chunks/DMA
            st_all = stallp.tile([P, NCH, 512], bf16, tag="stall")
            h_w = []
            ngrp = 2 * OH // 512 if 2 * OH > 512 else 1
            for hc in range(NCH // 4):
                hstage = hstp.tile([P, 4, 512], bf16, tag="hst")
                for sub in range(4):
                    nchunk = 4 * hc + sub
                    fsl = slice(P * nchunk, P * (nchunk + 1))
                    if ngrp == 1:
                        pt = ps_mm.tile([P, 512], f32, tag="mm")
                        for cc in range(CC):
                            nc.tensor.matmul(
                                out=pt[:, 0:2 * OH],
                                lhsT=fprev[:, cc, fsl],
                                rhs=wv[:, cc, :],
                                start=(cc == 0), stop=(cc == CC - 1))
                        nc.scalar.copy(hstage[:, sub, 0:OH], pt[:, 0:OH])
                        nc.scalar.copy(st_all[:, nchunk, 0:OH], pt[:, OH:2 * OH])
                    else:
                        pa = ps_mm.tile([P, 512], f32, tag="mm")
                        pb = ps_mm.tile([P, 512], f32, tag="mm")
                        for cc in range(CC):
                            nc.tensor.matmul(
                                out=pa[:], lhsT=fprev[:, cc, fsl],
                                rhs=wv[:, cc, 0:OH],
                                start=(cc == 0), stop=(cc == CC - 1))
                            nc.tensor.matmul(
                                out=pb[:], lhsT=fprev[:, cc, fsl],
                                rhs=wv[:, cc, OH:2 * OH],
                                start=(cc == 0), stop=(cc == CC - 1))
                        nc.scalar.copy(hstage[:, sub, 0:OH], pa[:])
                        nc.scalar.copy(st_all[:, nchunk, 0:OH], pb[:])
                wi = nc.sync.dma_start(
                    htbl[bi][4 * P * hc:4 * P * (hc + 1), :].rearrange(
                        "(s p) o -> p s o", p=P),
                    hstage[:, :, 0:OH])
                h_w.append(wi)

            if KSTOP == f"c1_{bi+1}":
                _bail()
                return
            # ---- C.2: gather -> e -> max + stats ----
            ste = ps_st.tile([1, 512], f32, tag="ste", name=f"ste{bi}")
            stq = ps_st.tile([1, 512], f32, tag="stq", name=f"stq{bi}")
            # fmax2: channel-chunk-major staging [P, OC, NCH, 128]
            fmax2 = fmaxp.tile([P, 4, NCH, P], bf16, tag="fmax")
            for qc in range(NCH):
                g = gpool.tile([P, K, 512], bf16, tag="g")
                for k in range(K):
                    gi = nc.gpsimd.indirect_dma_start(
                        out=g[:, k, 0:OH], out_offset=None, in_=htbl[bi],
                        in_offset=bass.IndirectOffsetOnAxis(
                            ap=idx_all[:, qc, k:k + 1], axis=0))
                    for wi in h_w:
                        add_dep_helper(gi.ins, wi.ins, reason="gather after htbl")

                e = epool.tile([P, K, 512], bf16, tag="e")
                stb = st_all[:, qc, 0:OH].unsqueeze(1).broadcast_to([P, K, OH])
                nc.vector.tensor_tensor(out=e[:, 0:K, 0:OH], in0=g[:, :, 0:OH],
                                        in1=stb, op=ALU.add)
                # max over K: two pair-merged steps
                m2 = epool.tile([P, 2, 512], bf16, tag="m2")
                nc.vector.tensor_tensor(out=m2[:, :, 0:OH], in0=e[:, 0:2, 0:OH],
                                        in1=e[:, 2:4, 0:OH], op=ALU.max)
                nc.vector.tensor_tensor(
                    out=fmax2[:, 0:OC, qc, :],
                    in0=m2[:, 0, 0:OH].rearrange("p (c o) -> p c o", o=P),
                    in1=m2[:, 1, 0:OH].rearrange("p (c o) -> p c o", o=P),
                    op=ALU.max)
                # e sum over K: strided reduce (innermost = K)
                esum = epool.tile([P, 512], bf16, tag="esum")
                with nc.allow_low_precision("4-term bf16 sum; block stats accumulate in f32 PSUM"):
                    nc.vector.tensor_reduce(
                        out=esum[:, 0:OH],
                        in_=e[:].rearrange("p k o -> p o k")[:, 0:OH, :],
                        axis=AX.X, op=ALU.add)
                # e^2 sums
                sqe = epool.tile([P, K, 512], bf16, tag="sqe")
                nc.scalar.activation(sqe[:, 0:K, 0:OH], e[:, 0:K, 0:OH], AF.Square)
                qsum = epool.tile([P, 512], bf16, tag="qsum")
                with nc.allow_low_precision("4-term bf16 sum; block stats accumulate in f32 PSUM"):
                    nc.vector.tensor_reduce(
                        out=qsum[:, 0:OH],
                        in_=sqe[:].rearrange("p k o -> p o k")[:, 0:OH, :],
                        axis=AX.X, op=ALU.add)
                # stats partition-reduction: accumulate in PSUM rows
                nc.tensor.matmul(out=ste[:, 0:OH], lhsT=ones_colb[:],
                                 rhs=esum[:, 0:OH], start=(qc == 0),
                                 stop=(qc == NCH - 1), skip_group_check=True)
                nc.tensor.matmul(out=stq[:, 0:OH], lhsT=ones_colb[:],
                                 rhs=qsum[:, 0:OH], start=(qc == 0),
                                 stop=(qc == NCH - 1), skip_group_check=True)

            if KSTOP == f"c2_{bi+1}":
                _bail()
                return
            # ---- C.3: group stats -> sc/bb rows on partition 0 ----
            gp = st1.tile([1, 2048], f32, tag="gp")
            nc.scalar.dma_start(gp[:], gnp[bi])
            combo = st1.tile([1, 2048], f32, tag="combo")  # [sc | bb] cols
            cnt = float(grp * N * K)
            gsum = st1.tile([1, 8], f32, tag="gsum")  # [mu*2 | e2m*2 | sd*2 | r*2]
            nc.vector.tensor_reduce(
                out=gsum[:, 0:2],
                in_=ste[:, 0:OH].rearrange("p (g c) -> p g c", g=2),
                axis=AX.X, op=ALU.add)
            nc.vector.tensor_reduce(
                out=gsum[:, 2:4],
                in_=stq[:, 0:OH].rearrange("p (g c) -> p g c", g=2),
                axis=AX.X, op=ALU.add)
            nc.vector.tensor_scalar_mul(gsum[:, 0:4], gsum[:, 0:4], 1.0 / cnt)
            musq = st1.tile([1, 2], f32, tag="musq")
            nc.vector.tensor_tensor(out=musq[:], in0=gsum[:, 0:2],
                                    in1=gsum[:, 0:2], op=ALU.mult)
            nc.vector.tensor_tensor(out=gsum[:, 2:4], in0=gsum[:, 2:4],
                                    in1=musq[:], op=ALU.subtract)
            nc.scalar.activation(gsum[:, 4:6], gsum[:, 2:4], AF.Sqrt,
                                 bias=eps_b[:, 0:1])
            nc.vector.reciprocal(gsum[:, 6:8], gsum[:, 4:6])
            rview = gsum[:, 6:8].unsqueeze(-1).broadcast_to([1, 2, grp])
            muv = gsum[:, 0:2].unsqueeze(-1).broadcast_to([1, 2, grp])
            scv = combo[:, 0:OH].rearrange("p (g c) -> p g c", g=2)
            bbv = combo[:, 1024:1024 + OH].rearrange("p (g c) -> p g c", g=2)
            gwv = gp[:, 0:OH].rearrange("p (g c) -> p g c", g=2)
            gbv = gp[:, 1024:1024 + OH].rearrange("p (g c) -> p g c", g=2)
            nc.vector.tensor_tensor(out=scv, in0=gwv, in1=rview, op=ALU.mult)
            nc.vector.tensor_tensor(out=bbv, in0=muv, in1=scv, op=ALU.mult)
            nc.vector.tensor_tensor(out=bbv, in0=gbv, in1=bbv, op=ALU.subtract)

            if KSTOP == f"c3_{bi+1}":
                _bail()
                return
            # ---- C.4: XBAR transpose per oc + wide Prelu -> agin DRAM ----
            dstw = dstwp.tile([P, 4, N], bf16, tag="dstw")
            for oc in range(OC):
                cps = ps_tr.tile([P, 2], f32, tag="tp")
                nc.tensor.transpose(out=cps[:, 0:1],
                                    in_=combo[:, P * oc:P * (oc + 1)],
                                    identity=idt[0:1, 0:1])
                nc.tensor.transpose(out=cps[:, 1:2],
                                    in_=combo[:, 1024 + P * oc:1024 + P * (oc + 1)],
                                    identity=idt[0:1, 0:1])
                col = st1.tile([P, 2], f32, tag="col")
                nc.scalar.copy(col[:], cps[:, 0:2])
                ftr = ftrp.tile([P, NCH, P], bf16, tag="ftr")
                nc.sync.dma_start_transpose(
                    ftr[:], fmax2[:, oc, :, :].rearrange("p s o -> p (s o)"))
                nc.scalar.activation(
                    dstw[:, oc, :], ftr[:].rearrange("p s o -> p (s o)"),
                    AF.Prelu, bias=col[:, 1:2], scale=col[:, 0:1],
                    alpha=alpha_col[:, 0:1])
            wi = nc.scalar.dma_start(
                agin[bi].rearrange("(c p) n -> p c n", p=P),
                dstw[:, 0:OC, :])

            # ---- pair AllGather ----
            if os.environ.get("NOCC", ""):
                # sim-only: stand-in DMAs with the same byte volume (the rust
                # cost model charges inter-chip rates for same-chip pairs)
                cc_i = nc.gpsimd.dma_start(agout[bi][0:OH, :], agin[bi])
                ci2 = nc.gpsimd.dma_start(agout[bi][OH:2 * OH, :], agin[bi])
                add_dep_helper(ci2.ins, wi.ins, reason="allgather after agin")
            else:
                cc_i = nc.gpsimd.collective_compute(
                    "AllGather", ALU.bypass, replica_groups=PAIRS,
                    ins=[agin[bi]], outs=[agout[bi]])
            add_dep_helper(cc_i.ins, wi.ins, reason="allgather after agin")
            ag_cc[bi] = cc_i

            # ---- load full f_next from agout (not needed after block 4) ----
            if bi < 3:
                CCn = 2 * OH // P
                fnext = fpool.tile([P, 4, N], bf16, tag="f", name=f"f{bi+1}")
                li = nc.scalar.dma_start(
                    fnext[:, 0:CCn, :],
                    agout[bi].rearrange("(c p) n -> p c n", p=P))
                add_dep_helper(li.ins, cc_i.ins, reason="fnext after allgather")
            else:
                fnext = None
            fprev = fnext
            if KSTOP == f"blk{bi+1}":
                _bail()
                return

        # ============ Final conv5 + GN5 + LeakyReLU (own DDH half) ==========
        w5t = wpool.tile([P, 19, DDH], bf16, tag="w")
        w5v = w5t[:]
        nc.sync.dma_start(w5v[:], w5a.rearrange("(c p) o -> p c o", p=P))
        nc.sync.dma_start(g5[:], gn5t.rearrange("(o p) s -> p o s", p=P))

        cmap = []
        for bi, (_, o) in enumerate(BLOCKS):
            for r in range(2 * o // P):
                cmap.append((bi, r))
        assert len(cmap) == 18

        out5 = fmaxp.tile([P, 4, NCH, P], bf16, tag="fmax")  # reuse slot bytes
        out5v = out5[:].rearrange("p a b c -> p (a b c)").bitcast(
            mybir.dt.float32).rearrange("p (c n) -> p c n", n=N)  # [P, 2, N] f32
        for half in range(2):
            hsl = slice(1024 * half, 1024 * (half + 1))
            pts = [[ps_mm.tile([P, 512], f32, tag="mm", name=f"pt5_{half}_{q}_{j}")
                    for j in range(2)] for q in range(2)]
            for cc in range(18):
                bi_, r_ = cmap[cc]
                fct = st3.tile([P, 1024], bf16, tag="fcl")
                li = nc.scalar.dma_start(fct[:], agout[bi_][P * r_:P * (r_ + 1), hsl])
                add_dep_helper(li.ins, ag_cc[bi_].ins, reason="fc after allgather")
                for o5 in range(2):
                    for q in range(2):
                        nc.tensor.matmul(out=pts[q][o5][:],
                                         lhsT=w5v[:, cc, P * o5:P * (o5 + 1)],
                                         rhs=fct[:, 512 * q:512 * (q + 1)],
                                         start=(cc == 0), stop=False)
            for o5 in range(2):
                for q in range(2):
                    qs = 2 * half + q
                    sl = slice(512 * qs, 512 * (qs + 1))
                    nc.tensor.matmul(out=pts[q][o5][:],
                                     lhsT=w5v[:, 18, P * o5:P * (o5 + 1)],
                                     rhs=ones_rhs[:], start=False, stop=True)
                    nc.scalar.activation(out5v[:, o5, sl], pts[q][o5][:],
                                         AF.Identity, accum_out=acc[:, o5, qs:qs + 1])
                    sq5 = st1.tile([P, 512], f32, tag="sq5")
                    nc.scalar.activation(sq5[:], pts[q][o5][:], AF.Square,
                                         accum_out=acc2[:, o5, qs:qs + 1])

        for o5 in range(2):
            s1 = st1.tile([P, 2], f32, tag="s5")
            nc.vector.tensor_reduce(out=s1[:, 0:1], in_=acc[:, o5, 0:4],
                                    axis=AX.X, op=ALU.add)
            nc.vector.tensor_reduce(out=s1[:, 1:2], in_=acc2[:, o5, 0:4],
                                    axis=AX.X, op=ALU.add)
            tot = ps_st.tile([1, 2], f32, tag="ste")
            nc.tensor.matmul(out=tot[:], lhsT=ones_col[:], rhs=s1[:],
                             start=True, stop=True)
            stot = st1.tile([1, 2], f32, tag="stot")
            nc.scalar.copy(stot[:], tot[:])
            bc = ps_st.tile([P, 2], f32, tag="stq")
            nc.tensor.matmul(out=bc[:], lhsT=ones_row[:], rhs=stot[:],
                             start=True, stop=True)
            stat = st1.tile([P, 4], f32, tag="stat")   # [mu, var, sd, r]
            nc.scalar.copy(stat[:, 0:2], bc[:])
            nc.vector.tensor_scalar_mul(stat[:, 0:2], stat[:, 0:2], 1.0 / (P * N))
            mu2 = st1.tile([P, 1], f32, tag="mu2")
            nc.vector.tensor_tensor(out=mu2[:], in0=stat[:, 0:1], in1=stat[:, 0:1],
                                    op=ALU.mult)
            nc.vector.tensor_tensor(out=stat[:, 1:2], in0=stat[:, 1:2], in1=mu2[:],
                                    op=ALU.subtract)
            nc.scalar.activation(stat[:, 2:3], stat[:, 1:2], AF.Sqrt,
                                 bias=eps_col[:, 0:1])
            nc.vector.reciprocal(stat[:, 3:4], stat[:, 2:3])
            sc5 = st1.tile([P, 2], f32, tag="sc5")
            nc.vector.tensor_tensor(out=sc5[:, 0:1], in0=g5[:, o5, 0:1],
                                    in1=stat[:, 3:4], op=ALU.mult)
            nc.vector.tensor_tensor(out=sc5[:, 1:2], in0=stat[:, 0:1],
                                    in1=sc5[:, 0:1], op=ALU.mult)
            nc.vector.tensor_tensor(out=sc5[:, 1:2], in0=g5[:, o5, 1:2],
                                    in1=sc5[:, 1:2], op=ALU.subtract)
            for qs in range(N // 512):
                sl = slice(512 * qs, 512 * (qs + 1))
                ot = st3.tile([P, 512], f32, tag="sq5o")
                nc.scalar.activation(ot[:], out5v[:, o5, sl], AF.Prelu,
                                     bias=sc5[:, 1:2], scale=sc5[:, 0:1],
                                     alpha=alpha_col[:, 0:1])
                nc.sync.dma_start(outT[P * o5:P * (o5 + 1), sl], ot[:])


def _host_prep(inputs):
    smp = np.asarray(inputs["sampled"], np.float32)
    ctr = np.asarray(inputs["center"], np.float32)
    act = np.asarray(inputs["action"], np.float32)
    w_in = np.asarray(inputs["w_in"], np.float32)
    b_in = np.asarray(inputs["b_in"], np.float32)
    ws = [np.asarray(inputs[f"w{i}"], np.float32) for i in (1, 2, 3, 4)]
    gws = [np.asarray(inputs[f"g{i}w"], np.float32) for i in (1, 2, 3, 4)]
    gbs = [np.asarray(inputs[f"g{i}b"], np.float32) for i in (1, 2, 3, 4)]
    w5 = np.asarray(inputs["w5"], np.float32)
    b5 = np.asarray(inputs["b5"], np.float32)
    g5w = np.asarray(inputs["g5w"], np.float32)
    g5b = np.asarray(inputs["g5b"], np.float32)

    shared = {}
    shared["wina"] = np.concatenate([w_in.T, b_in[None, :]], 0).astype(BF16)

    in_maps = []
    for core in range(8):
        b, h = core // 2, core % 2
        m = dict(shared)
        for i, (C, OH) in enumerate(BLOCKS):
            w = ws[i]
            hs = slice(h * OH, (h + 1) * OH)
            wl, wr = w[hs, :C], w[hs, C:]
            m[f"wstk{i+1}"] = np.concatenate([wl.T, (wr - wl).T], 1).astype(BF16)
            g = np.zeros((1, 2048), np.float32)
            g[0, :OH] = gws[i][hs]
            g[0, 1024:1024 + OH] = gbs[i][hs]
            m[f"gnp{i+1}"] = g
        h5 = slice(h * DDH, (h + 1) * DDH)
        w5a = np.zeros((19 * P, DDH), np.float32)
        w5a[:2304] = w5[h5].T
        w5a[2304] = b5[h5]
        m["w5a"] = w5a.astype(BF16)
        m["gn5t"] = np.stack([g5w[h5], g5b[h5]], 1).astype(np.float32)

        x = np.concatenate([smp[b].T, np.repeat(act[b][:, None], N, 1),
                            np.ones((1, N), np.float32)], 0)
        coor = ctr[b].T
        sq = (coor * coor).sum(0).astype(np.float32)
        m["xaug"] = x.astype(BF16)
        knl = np.concatenate([coor, np.ones((1, N), np.float32)], 0)
        m["knnl"] = knl[:, h * (N // 2):(h + 1) * (N // 2)].copy()
        m["knnr"] = np.concatenate([2.0 * coor, -sq[None, :]], 0)
        in_maps.append(m)
    return in_maps


def kernel(**inputs):
    for i in (1, 2, 3, 4):
        assert np.all(np.asarray(inputs[f"g{i}w"]) >= 0), \
            "kernel assumes non-negative GN weights (max/LeakyReLU commute)"
    if "nc" not in _cache:
        _cache["nc"] = _build_nc()
    nc = _cache["nc"]
    in_maps = _host_prep(inputs)
    res = run_bass_kernel_spmd(nc, in_maps, core_ids=list(range(8)))
    out = np.empty((B, N, DD), np.float32)
    for b in range(B):
        out[b, :, 0:DDH] = res.results[2 * b]["outT"].T
        out[b, :, DDH:DD] = res.results[2 * b + 1]["outT"].T
    return out


def _get_fast(in_maps):
    if "fast" in _cache:
        return _cache["fast"]
    import jax
    from jax.sharding import Mesh, PartitionSpec, NamedSharding
    from jax.experimental.shard_map import shard_map
    from concourse import bass2jax
    import concourse.mybir as mb
    nc = _cache["nc"]
    bass2jax.install_neuronx_cc_hook()
    partition_name = nc.partition_id_tensor.name if nc.partition_id_tensor else None
    in_names, out_names, out_avals, zero_outs = [], [], [], []
    for alloc in nc.m.functions[0].allocations:
        if not isinstance(alloc, mb.MemoryLocationSet):
            continue
        name = alloc.memorylocations[0].name
        if alloc.kind == "ExternalInput":
            if name != partition_name:
                in_names.append(name)
        elif alloc.kind == "ExternalOutput":
            out_names.append(name)
            shape = tuple(alloc.tensor_shape)
            dtype = mb.dt.np(alloc.dtype)
            out_avals.append(jax.core.ShapedArray(shape, dtype))
            zero_outs.append(np.zeros(shape, dtype))
    n_params = len(in_names)
    all_in = in_names + out_names + ([partition_name] if partition_name else [])

    def _body(*args):
        operands = list(args)
        if partition_name is not None:
            operands.append(bass2jax.partition_id_tensor())
        outs = bass2jax._bass_exec_p.bind(
            *operands, out_avals=tuple(out_avals), in_names=tuple(all_in),
            out_names=tuple(out_names), lowering_input_output_aliases=(),
            sim_require_finite=True, sim_require_nnan=True, nc=nc)
        return tuple(outs)

    devices = jax.devices()[:8]
    mesh = Mesh(np.asarray(devices), ("core",))
    fn = jax.jit(shard_map(_body, mesh=mesh,
                           in_specs=(PartitionSpec("core"),) * (n_params + len(out_names)),
                           out_specs=(PartitionSpec("core"),) * len(out_names),
                           check_rep=False), keep_unused=True)
    sharding = NamedSharding(mesh, PartitionSpec("core"))
    _cache["fast"] = (fn, in_names, out_names, zero_outs, sharding)
    return _cache["fast"]


def timed_run(inputs, reps=10):
    import time as _t
    import jax
    if "nc" not in _cache:
        _cache["nc"] = _build_nc()
    in_maps = _host_prep(inputs)
    fn, in_names, out_names, zero_outs, sharding = _get_fast(in_maps)
    concat_in = [np.concatenate([np.asarray(m[nm]) for m in in_maps], 0)
                 for nm in in_names]
    concat_zeros = [np.zeros((8 * z.shape[0], *z.shape[1:]), z.dtype)
                    for z in zero_outs]
    dev = [jax.device_put(a, sharding) for a in concat_in + concat_zeros]
    r = fn(*dev); jax.block_until_ready(r)
    times = []
    for _ in range(reps):
        t0 = _t.perf_counter()
        r = fn(*dev)
        jax.block_until_ready(r)
        times.append(_t.perf_counter() - t0)
    oi = out_names.index("outT")
    arr = np.asarray(r[oi]).reshape(8, DDH, N)
    out = np.empty((B, N, DD), np.float32)
    for b in range(B):
        out[b, :, 0:DDH] = arr[2 * b].T
        out[b, :, DDH:DD] = arr[2 * b + 1].T
    return times, out


# revision 12
# speedup vs baseline: 3.3065x; 3.3065x over previous
"""Trainium2 Bass kernel for nn_CGCNNDynamics (Point-BERT DGCNN dynamics head).

V2: pair-split (2 cores per batch element, channel halves at GN group
boundaries) + instruction-count-oriented restructure:
 - one combined [hT | sT] matmul pass per point chunk
 - one 4-neighbor indirect gather per query chunk (multi-index offset AP)
 - edge values e = g + s formed explicitly; GN stats = sum(e), sum(e^2)
   accumulated across the whole block in two [1, OH] PSUM rows via
   ones-stationary matmuls (2 per query chunk)
 - channel-major staging + XBAR dma_start_transpose (per-slice 128x128)
   replaces per-chunk PE transposes; one wide Prelu per output-channel chunk
 - merged DMAs (4-chunk htbl writes, 1 DMA per block for agin/fnext)
 - pair AllGather of bf16 channel halves between blocks
"""
import sys, os
sys.path.insert(0, "/opt/trn_rl_repo")
KSTOP = os.environ.get("KSTOP", "")
import contextlib
import numpy as np
import ml_dtypes

import concourse.bass as bass
import concourse.bacc as bacc
import concourse.mybir as mybir
import concourse.tile as tile
from concourse.tile import add_dep_helper
from concourse.bass_utils import run_bass_kernel_spmd

BF16 = ml_dtypes.bfloat16
P = 128
B, N, TD, AD, DD = 4, 2048, 256, 8, 512
DDH = DD // 2
CIN = TD + AD          # 264
K = 4
EPS = 1e-5
ALPHA = 0.2
NCH = N // P           # 16
BLOCKS = [(128, 128), (256, 256), (512, 256), (512, 512)]   # (C_full, O_half)
PAIRS = [[0, 1], [2, 3], [4, 5], [6, 7]]

_cache = {}


def _build_nc():
    nc = bacc.Bacc("TRN2", target_bir_lowering=False, debug=False,
                   enable_asserts=False, num_devices=8)
    f32, bf16, u32 = mybir.dt.float32, mybir.dt.bfloat16, mybir.dt.uint32

    xaug = nc.dram_tensor("xaug", [CIN + 1, N], bf16, kind="ExternalInput").ap()
    wina = nc.dram_tensor("wina", [CIN + 1, P], bf16, kind="ExternalInput").ap()
    knnl = nc.dram_tensor("knnl", [4, N // 2], f32, kind="ExternalInput").ap()
    knnr = nc.dram_tensor("knnr", [4, N], f32, kind="ExternalInput").ap()
    wstk = [nc.dram_tensor(f"wstk{i+1}", [c, 2 * o], bf16, kind="ExternalInput").ap()
            for i, (c, o) in enumerate(BLOCKS)]
    gnp = [nc.dram_tensor(f"gnp{i+1}", [1, 2048], f32, kind="ExternalInput").ap()
           for i in range(4)]
    w5a = nc.dram_tensor("w5a", [19 * P, DDH], bf16, kind="ExternalInput").ap()
    gn5t = nc.dram_tensor("gn5t", [DDH, 2], f32, kind="ExternalInput").ap()
    outT = nc.dram_tensor("outT", [DDH, N], f32, kind="ExternalOutput").ap()

    htbl = [nc.dram_tensor(f"htbl{i+1}", [N, o], bf16, kind="Internal").ap()
            for i, (_, o) in enumerate(BLOCKS)]
    agin = [nc.dram_tensor(f"agin{i+1}", [o, N], bf16, kind="Internal").ap()
            for i, (_, o) in enumerate(BLOCKS)]
    agout = [nc.dram_tensor(f"agout{i+1}", [2 * o, N], bf16, kind="Internal").ap()
             for i, (_, o) in enumerate(BLOCKS)]
    idxin = nc.dram_tensor("idxin", [P, 64], mybir.dt.uint32, kind="Internal").ap()
    idxout = nc.dram_tensor("idxout", [2 * P, 64], mybir.dt.uint32,
                            kind="Internal").ap()

    with tile.TileContext(nc) as tc:
        _emit(nc, tc, xaug, wina, knnl, knnr, wstk, gnp, w5a, gn5t,
              outT, htbl, agin, agout, idxin, idxout)
    nc.compile()
    return nc


def _emit(nc, tc, xaug, wina, knnl, knnr, wstk, gnp, w5a, gn5t,
          outT, htbl, agin, agout, idxin, idxout):
    f32, bf16, u32 = mybir.dt.float32, mybir.dt.bfloat16, mybir.dt.uint32
    AX, ALU, AF = mybir.AxisListType, mybir.AluOpType, mybir.ActivationFunctionType
    from concourse.masks import make_identity

    ctx = contextlib.ExitStack()
    with ctx:
        fpool = ctx.enter_context(tc.tile_pool(name="fpool", bufs=2))
        wpool = ctx.enter_context(tc.tile_pool(name="wpool", bufs=1))
        gpool = ctx.enter_context(tc.tile_pool(name="gpool", bufs=3))
        ftrp = ctx.enter_context(tc.tile_pool(name="ftrp", bufs=2))
        epool = ctx.enter_context(tc.tile_pool(name="epool", bufs=3))
        fmaxp = ctx.enter_context(tc.tile_pool(name="fmaxp", bufs=1))
        stallp = ctx.enter_context(tc.tile_pool(name="stallp", bufs=1))
        dstwp = ctx.enter_context(tc.tile_pool(name="dstwp", bufs=1))
        hstp = ctx.enter_context(tc.tile_pool(name="hstp", bufs=1))
        st1 = ctx.enter_context(tc.tile_pool(name="st1", bufs=1))
        st3 = ctx.enter_context(tc.tile_pool(name="st3", bufs=2))
        smalls = ctx.enter_context(tc.tile_pool(name="smalls", bufs=1))
        ps_mm = ctx.enter_context(tc.tile_pool(name="ps_mm", bufs=4, space="PSUM"))
        ps_st = ctx.enter_context(tc.tile_pool(name="ps_st", bufs=1, space="PSUM"))
        ps_tr = ctx.enter_context(tc.tile_pool(name="ps_tr", bufs=1, space="PSUM"))

        # ---- constants ----
        idt = smalls.tile([P, P], f32, tag="idt")
        make_identity(nc, idt[:])
        ones_col = smalls.tile([P, 1], f32, tag="ones_col")
        nc.vector.memset(ones_col[:], 1.0)
        ones_colb = smalls.tile([P, 1], bf16, tag="ones_colb")
        nc.vector.memset(ones_colb[:], 1.0)
        ones_row = smalls.tile([1, P], f32, tag="ones_row")
        nc.vector.memset(ones_row[:], 1.0)
        ones_rhs = smalls.tile([P, 512], bf16, tag="ones_rhs")
        nc.vector.memset(ones_rhs[:], 1.0)
        alpha_col = smalls.tile([P, 1], f32, tag="alpha_col")
        nc.vector.memset(alpha_col[:], ALPHA)
        eps_b = smalls.tile([1, 1], f32, tag="eps_b")
        nc.vector.memset(eps_b[:], EPS)
        eps_col = smalls.tile([P, 1], f32, tag="eps_col")
        nc.vector.memset(eps_col[:], EPS)
        idx_all = smalls.tile([P, NCH, 8], u32, tag="idx")
        top8v = smalls.tile([P, 8], f32, tag="top8v")
        wia = smalls.tile([P, 3, P], bf16, tag="wia")
        g5 = smalls.tile([P, 2, 2], f32, tag="g5")
        acc = smalls.tile([P, 2, 8], f32, tag="acc")
        acc2 = smalls.tile([P, 2, 8], f32, tag="acc2")

        def _bail():
            z = st1.tile([P, 512], f32, tag="sq5", name="bailz")
            nc.vector.memset(z[:], 0.0)
            for o5_ in range(DDH // P):
                for qs_ in range(N // 512):
                    nc.sync.dma_start(
                        outT[P * o5_:P * (o5_ + 1), 512 * qs_:512 * (qs_ + 1)], z[:])
        # ============ Phase A: conv_in -> f0 (chan-part bf16) ============
        xg = wpool.tile([P, 3, N], bf16, tag="w")
        xgv = xg[:]
        nc.sync.dma_start(xgv[:, 0, :], xaug[0:P, :])
        nc.sync.dma_start(xgv[:, 1, :], xaug[P:2 * P, :])
        nc.sync.dma_start(xgv[0:9, 2, :], xaug[2 * P:CIN + 1, :])
        nc.sync.dma_start(wia[:, 0, :], wina[0:P, :])
        nc.sync.dma_start(wia[:, 1, :], wina[P:2 * P, :])
        nc.sync.dma_start(wia[0:9, 2, :], wina[2 * P:CIN + 1, :])

        f0 = fpool.tile([P, 4, N], bf16, tag="f", name="f0")
        for qs in range(N // 512):
            pt = ps_mm.tile([P, 512], f32, tag="mm")
            sl = slice(512 * qs, 512 * (qs + 1))
            nc.tensor.matmul(out=pt[:], lhsT=wia[:, 0, :], rhs=xgv[:, 0, sl],
                             start=True, stop=False)
            nc.tensor.matmul(out=pt[:], lhsT=wia[:, 1, :], rhs=xgv[:, 1, sl],
                             start=False, stop=False)
            nc.tensor.matmul(out=pt[:], lhsT=wia[0:9, 2, :], rhs=xgv[0:9, 2, sl],
                             start=False, stop=True)
            nc.scalar.copy(f0[:, 0, sl], pt[:])

        if KSTOP == "a":
            _bail()
            return
        # ============ Phase B: KNN top-4 indices ============
        kl = smalls.tile([4, N // 2], f32, tag="kl")
        nc.sync.dma_start(kl[:], knnl)
        kr = smalls.tile([4, N], f32, tag="kr")
        nc.sync.dma_start(kr[:], knnr)
        for qc in range(NCH // 2):
            dsb = fmaxp.tile([P, 4, NCH, P], bf16, tag="fmax", name=f"dv{qc}")
            dview = dsb[:].rearrange("p a b c -> p (a b c)").bitcast(f32)[:, 0:2048]
            for js in range(N // 512):
                pt = ps_mm.tile([P, 512], f32, tag="mm")
                nc.tensor.matmul(out=pt[:], lhsT=kl[:, P * qc:P * (qc + 1)],
                                 rhs=kr[:, 512 * js:512 * (js + 1)],
                                 start=True, stop=True)
                nc.scalar.copy(dview[:, 512 * js:512 * (js + 1)], pt[:])
            nc.vector.max(out=top8v[:], in_=dview)
            nc.vector.max_index(out=idx_all[:, qc, :], in_max=top8v[:],
                                in_values=dview)
        # exchange halves: own idx -> slot h of idxout
        iw = nc.sync.dma_start(idxin, idx_all[:, 0:NCH // 2, :].rearrange(
            "p c o -> p (c o)"))
        if os.environ.get("NOCC", ""):
            icc = nc.gpsimd.dma_start(idxout[0:P, :], idxin)
            ic2 = nc.gpsimd.dma_start(idxout[P:2 * P, :], idxin)
            add_dep_helper(ic2.ins, iw.ins, reason="idx ag after idxin")
        else:
            icc = nc.gpsimd.collective_compute(
                "AllGather", ALU.bypass, replica_groups=PAIRS,
                ins=[idxin], outs=[idxout])
        add_dep_helper(icc.ins, iw.ins, reason="idx ag after idxin")
        il1 = nc.sync.dma_start(idx_all[:, 0:NCH // 2, :].rearrange(
            "p c o -> p (c o)"), idxout[0:P, :])
        il2 = nc.sync.dma_start(idx_all[:, NCH // 2:NCH, :].rearrange(
            "p c o -> p (c o)"), idxout[P:2 * P, :])
        add_dep_helper(il1.ins, icc.ins, reason="idx load after ag")
        add_dep_helper(il2.ins, icc.ins, reason="idx load after ag")

        if KSTOP == "b":
            _bail()
            return
        # ============ Edge blocks ============
        fprev = f0
        prevCC = 1
        ag_cc = [None] * 4
        for bi, (C, OH) in enumerate(BLOCKS):
            CC = C // P
            OC = OH // P
            grp = OH // 2
            # one DMA for the whole weight stack [C, 2*OH] -> [P, CC, 2*OH]
            wk = wpool.tile([P, 8, 512], bf16, tag="w")
            wv = wk[:].rearrange("p c o -> p (c o)")[:, 0:CC * 2 * OH].rearrange(
                "p (c o) -> p c o", o=2 * OH)
            nc.scalar.dma_start(
                wv[:], wstk[bi].rearrange("(c p) o -> p c o", p=P))

            # ---- C.1: combined [hT | sT] pass ----
            # st_all: sT stash bf16 [P, NCH, OH]; htbl written 4 chunks/DMA
            st_all = stallp.tile([P, NCH, 512], bf16, tag="stall")
            h_w = []
            ngrp = 2 * OH // 512 if 2 * OH > 512 else 1
            for hc in range(NCH // 4):
                hstage = hstp.tile([P, 4, 512], bf16, tag="hst")
                for sub in range(4):
                    nchunk = 4 * hc + sub
                    fsl = slice(P * nchunk, P * (nchunk + 1))
                    if ngrp == 1:
                        pt = ps_mm.tile([P, 512], f32, tag="mm")
                        for cc in range(CC):
                            nc.tensor.matmul(
                                out=pt[:, 0:2 * OH],
                                lhsT=fprev[:, cc, fsl],
                                rhs=wv[:, cc, :],
                                start=(cc == 0), stop=(cc == CC - 1))
                        nc.scalar.copy(hstage[:, sub, 0:OH], pt[:, 0:OH])
                        nc.scalar.copy(st_all[:, nchunk, 0:OH], pt[:, OH:2 * OH])
                    else:
                        pa = ps_mm.tile([P, 512], f32, tag="mm")
                        pb = ps_mm.tile([P, 512], f32, tag="mm")
                        for cc in range(CC):
                            nc.tensor.matmul(
                                out=pa[:], lhsT=fprev[:, cc, fsl],
                                rhs=wv[:, cc, 0:OH],
                                start=(cc == 0), stop=(cc == CC - 1))
                            nc.tensor.matmul(
                                out=pb[:], lhsT=fprev[:, cc, fsl],
                                rhs=wv[:, cc, OH:2 * OH],
                                start=(cc == 0), stop=(cc == CC - 1))
                        nc.scalar.copy(hstage[:, sub, 0:OH], pa[:])
                        nc.scalar.copy(st_all[:, nchunk, 0:OH], pb[:])
                wi = nc.sync.dma_start(
                    htbl[bi][4 * P * hc:4 * P * (hc + 1), :].rearrange(
                        "(s p) o -> p s o", p=P),
                    hstage[:, :, 0:OH])
                h_w.append(wi)

            if KSTOP == f"c1_{bi+1}":
                _bail()
                return
            # ---- C.2: gather -> e -> max + stats ----
            ste = ps_st.tile([1, 512], f32, tag="ste", name=f"ste{bi}")
            stq = ps_st.tile([1, 512], f32, tag="stq", name=f"stq{bi}")
            # fmax2: channel-chunk-major staging [P, OC, NCH, 128]
            fmax2 = fmaxp.tile([P, 4, NCH, P], bf16, tag="fmax")
            for qc in range(NCH):
                g = gpool.tile([P, K, 512], bf16, tag="g")
                for k in range(K):
                    gi = nc.gpsimd.indirect_dma_start(
                        out=g[:, k, 0:OH], out_offset=None, in_=htbl[bi],
                        in_offset=bass.IndirectOffsetOnAxis(
                            ap=idx_all[:, qc, k:k + 1], axis=0))
                    for wi in h_w:
                        add_dep_helper(gi.ins, wi.ins, reason="gather after htbl")

                e = epool.tile([P, K, 512], bf16, tag="e")
                stb = st_all[:, qc, 0:OH].unsqueeze(1).broadcast_to([P, K, OH])
                nc.vector.tensor_tensor(out=e[:, 0:K, 0:OH], in0=g[:, :, 0:OH],
                                        in1=stb, op=ALU.add)
                # max over K: two pair-merged steps
                m2 = epool.tile([P, 2, 512], bf16, tag="m2")
                nc.vector.tensor_tensor(out=m2[:, :, 0:OH], in0=e[:, 0:2, 0:OH],
                                        in1=e[:, 2:4, 0:OH], op=ALU.max)
                nc.vector.tensor_tensor(
                    out=fmax2[:, 0:OC, qc, :],
                    in0=m2[:, 0, 0:OH].rearrange("p (c o) -> p c o", o=P),
                    in1=m2[:, 1, 0:OH].rearrange("p (c o) -> p c o", o=P),
                    op=ALU.max)
                # e sums: contiguous pair-merged adds on DVE
                s2 = epool.tile([P, 2, 512], bf16, tag="s2")
                nc.vector.tensor_tensor(out=s2[:, :, 0:OH], in0=e[:, 0:2, 0:OH],
                                        in1=e[:, 2:4, 0:OH], op=ALU.add)
                esum = epool.tile([P, 512], bf16, tag="esum")
                nc.vector.tensor_tensor(out=esum[:, 0:OH], in0=s2[:, 0, 0:OH],
                                        in1=s2[:, 1, 0:OH], op=ALU.add)
                # e^2 sums
                sqe = epool.tile([P, K, 512], bf16, tag="sqe")
                nc.scalar.activation(sqe[:, 0:K, 0:OH], e[:, 0:K, 0:OH], AF.Square)
                q2 = epool.tile([P, 2, 512], bf16, tag="q2")
                nc.vector.tensor_tensor(out=q2[:, :, 0:OH], in0=sqe[:, 0:2, 0:OH],
                                        in1=sqe[:, 2:4, 0:OH], op=ALU.add)
                qsum = epool.tile([P, 512], bf16, tag="qsum")
                nc.vector.tensor_tensor(out=qsum[:, 0:OH], in0=q2[:, 0, 0:OH],
                                        in1=q2[:, 1, 0:OH], op=ALU.add)
                # stats partition-reduction: accumulate in PSUM rows
                nc.tensor.matmul(out=ste[:, 0:OH], lhsT=ones_colb[:],
                                 rhs=esum[:, 0:OH], start=(qc == 0),
                                 stop=(qc == NCH - 1), skip_group_check=True)
                nc.tensor.matmul(out=stq[:, 0:OH], lhsT=ones_colb[:],
                                 rhs=qsum[:, 0:OH], start=(qc == 0),
                                 stop=(qc == NCH - 1), skip_group_check=True)

            if KSTOP == f"c2_{bi+1}":
                _bail()
                return
            # ---- C.3: group stats -> sc/bb rows on partition 0 ----
            gp = st1.tile([1, 2048], f32, tag="gp")
            nc.scalar.dma_start(gp[:], gnp[bi])
            combo = st1.tile([1, 2048], f32, tag="combo")  # [sc | bb] cols
            cnt = float(grp * N * K)
            gsum = st1.tile([1, 8], f32, tag="gsum")  # [mu*2 | e2m*2 | sd*2 | r*2]
            nc.vector.tensor_reduce(
                out=gsum[:, 0:2],
                in_=ste[:, 0:OH].rearrange("p (g c) -> p g c", g=2),
                axis=AX.X, op=ALU.add)
            nc.vector.tensor_reduce(
                out=gsum[:, 2:4],
                in_=stq[:, 0:OH].rearrange("p (g c) -> p g c", g=2),
                axis=AX.X, op=ALU.add)
            nc.vector.tensor_scalar_mul(gsum[:, 0:4], gsum[:, 0:4], 1.0 / cnt)
            musq = st1.tile([1, 2], f32, tag="musq")
            nc.vector.tensor_tensor(out=musq[:], in0=gsum[:, 0:2],
                                    in1=gsum[:, 0:2], op=ALU.mult)
            nc.vector.tensor_tensor(out=gsum[:, 2:4], in0=gsum[:, 2:4],
                                    in1=musq[:], op=ALU.subtract)
            nc.scalar.activation(gsum[:, 4:6], gsum[:, 2:4], AF.Sqrt,
                                 bias=eps_b[:, 0:1])
            nc.vector.reciprocal(gsum[:, 6:8], gsum[:, 4:6])
            rview = gsum[:, 6:8].unsqueeze(-1).broadcast_to([1, 2, grp])
            muv = gsum[:, 0:2].unsqueeze(-1).broadcast_to([1, 2, grp])
            scv = combo[:, 0:OH].rearrange("p (g c) -> p g c", g=2)
            bbv = combo[:, 1024:1024 + OH].rearrange("p (g c) -> p g c", g=2)
            gwv = gp[:, 0:OH].rearrange("p (g c) -> p g c", g=2)
            gbv = gp[:, 1024:1024 + OH].rearrange("p (g c) -> p g c", g=2)
            nc.vector.tensor_tensor(out=scv, in0=gwv, in1=rview, op=ALU.mult)
            nc.vector.tensor_tensor(out=bbv, in0=muv, in1=scv, op=ALU.mult)
            nc.vector.tensor_tensor(out=bbv, in0=gbv, in1=bbv, op=ALU.subtract)

            if KSTOP == f"c3_{bi+1}":
                _bail()
                return
            # ---- C.4: XBAR transpose per oc + wide Prelu -> agin DRAM ----
            dstw = dstwp.tile([P, 4, N], bf16, tag="dstw")
            for oc in range(OC):
                cps = ps_tr.tile([P, 2], f32, tag="tp")
                nc.tensor.transpose(out=cps[:, 0:1],
                                    in_=combo[:, P * oc:P * (oc + 1)],
                                    identity=idt[0:1, 0:1])
                nc.tensor.transpose(out=cps[:, 1:2],
                                    in_=combo[:, 1024 + P * oc:1024 + P * (oc + 1)],
                                    identity=idt[0:1, 0:1])
                col = st1.tile([P, 2], f32, tag="col")
                nc.scalar.copy(col[:], cps[:, 0:2])
                ftr = ftrp.tile([P, NCH, P], bf16, tag="ftr")
                nc.sync.dma_start_transpose(
                    ftr[:], fmax2[:, oc, :, :].rearrange("p s o -> p (s o)"))
                nc.scalar.activation(
                    dstw[:, oc, :], ftr[:].rearrange("p s o -> p (s o)"),
                    AF.Prelu, bias=col[:, 1:2], scale=col[:, 0:1],
                    alpha=alpha_col[:, 0:1])
            wi = nc.scalar.dma_start(
                agin[bi].rearrange("(c p) n -> p c n", p=P),
                dstw[:, 0:OC, :])

            # ---- pair AllGather ----
            if os.environ.get("NOCC", ""):
                # sim-only: stand-in DMAs with the same byte volume (the rust
                # cost model charges inter-chip rates for same-chip pairs)
                cc_i = nc.gpsimd.dma_start(agout[bi][0:OH, :], agin[bi])
                ci2 = nc.gpsimd.dma_start(agout[bi][OH:2 * OH, :], agin[bi])
                add_dep_helper(ci2.ins, wi.ins, reason="allgather after agin")
            else:
                cc_i = nc.gpsimd.collective_compute(
                    "AllGather", ALU.bypass, replica_groups=PAIRS,
                    ins=[agin[bi]], outs=[agout[bi]])
            add_dep_helper(cc_i.ins, wi.ins, reason="allgather after agin")
            ag_cc[bi] = cc_i

            # ---- load full f_next from agout (not needed after block 4) ----
            if bi < 3:
                CCn = 2 * OH // P
                fnext = fpool.tile([P, 4, N], bf16, tag="f", name=f"f{bi+1}")
                li = nc.scalar.dma_start(
                    fnext[:, 0:CCn, :],
                    agout[bi].rearrange("(c p) n -> p c n", p=P))
                add_dep_helper(li.ins, cc_i.ins, reason="fnext after allgather")
            else:
                fnext = None
            fprev = fnext
            if KSTOP == f"blk{bi+1}":
                _bail()
                return

        # ============ Final conv5 + GN5 + LeakyReLU (own DDH half) ==========
        w5t = wpool.tile([P, 19, DDH], bf16, tag="w")
        w5v = w5t[:]
        nc.sync.dma_start(w5v[:], w5a.rearrange("(c p) o -> p c o", p=P))
        nc.sync.dma_start(g5[:], gn5t.rearrange("(o p) s -> p o s", p=P))

        cmap = []
        for bi, (_, o) in enumerate(BLOCKS):
            for r in range(2 * o // P):
                cmap.append((bi, r))
        assert len(cmap) == 18

        out5 = fmaxp.tile([P, 4, NCH, P], bf16, tag="fmax")  # reuse slot bytes
        out5v = out5[:].rearrange("p a b c -> p (a b c)").bitcast(
            mybir.dt.float32).rearrange("p (c n) -> p c n", n=N)  # [P, 2, N] f32
        for half in range(2):
            hsl = slice(1024 * half, 1024 * (half + 1))
            pts = [[ps_mm.tile([P, 512], f32, tag="mm", name=f"pt5_{half}_{q}_{j}")
                    for j in range(2)] for q in range(2)]
            for cc in range(18):
                bi_, r_ = cmap[cc]
                fct = st3.tile([P, 1024], bf16, tag="fcl")
                li = nc.scalar.dma_start(fct[:], agout[bi_][P * r_:P * (r_ + 1), hsl])
                add_dep_helper(li.ins, ag_cc[bi_].ins, reason="fc after allgather")
                for o5 in range(2):
                    for q in range(2):
                        nc.tensor.matmul(out=pts[q][o5][:],
                                         lhsT=w5v[:, cc, P * o5:P * (o5 + 1)],
                                         rhs=fct[:, 512 * q:512 * (q + 1)],
                                         start=(cc == 0), stop=False)
            for o5 in range(2):
                for q in range(2):
                    qs = 2 * half + q
                    sl = slice(512 * qs, 512 * (qs + 1))
                    nc.tensor.matmul(out=pts[q][o5][:],
                                     lhsT=w5v[:, 18, P * o5:P * (o5 + 1)],
                                     rhs=ones_rhs[:], start=False, stop=True)
                    nc.scalar.activation(out5v[:, o5, sl], pts[q][o5][:],
                                         AF.Identity, accum_out=acc[:, o5, qs:qs + 1])
                    sq5 = st1.tile([P, 512], f32, tag="sq5")
                    nc.scalar.activation(sq5[:], pts[q][o5][:], AF.Square,
                                         accum_out=acc2[:, o5, qs:qs + 1])

        for o5 in range(2):
            s1 = st1.tile([P, 2], f32, tag="s5")
            nc.vector.tensor_reduce(out=s1[:, 0:1], in_=acc[:, o5, 0:4],
                                    axis=AX.X, op=ALU.add)
            nc.vector.tensor_reduce(out=s1[:, 1:2], in_=acc2[:, o5, 0:4],
                                    axis=AX.X, op=ALU.add)
            tot = ps_st.tile([1, 2], f32, tag="ste")
            nc.tensor.matmul(out=tot[:], lhsT=ones_col[:], rhs=s1[:],
                             start=True, stop=True)
            stot = st1.tile([1, 2], f32, tag="stot")
            nc.scalar.copy(stot[:], tot[:])
            bc = ps_st.tile([P, 2], f32, tag="stq")
            nc.tensor.matmul(out=bc[:], lhsT=ones_row[:], rhs=stot[:],
                             start=True, stop=True)
            stat = st1.tile([P, 4], f32, tag="stat")   # [mu, var, sd, r]
            nc.scalar.copy(stat[:, 0:2], bc[:])
            nc.vector.tensor_scalar_mul(stat[:, 0:2], stat[:, 0:2], 1.0 / (P * N))
            mu2 = st1.tile([P, 1], f32, tag="mu2")
            nc.vector.tensor_tensor(out=mu2[:], in0=stat[:, 0:1], in1=stat[:, 0:1],
                                    op=ALU.mult)
            nc.vector.tensor_tensor(out=stat[:, 1:2], in0=stat[:, 1:2], in1=mu2[:],
                                    op=ALU.subtract)
            nc.scalar.activation(stat[:, 2:3], stat[:, 1:2], AF.Sqrt,
                                 bias=eps_col[:, 0:1])
            nc.vector.reciprocal(stat[:, 3:4], stat[:, 2:3])
            sc5 = st1.tile([P, 2], f32, tag="sc5")
            nc.vector.tensor_tensor(out=sc5[:, 0:1], in0=g5[:, o5, 0:1],
                                    in1=stat[:, 3:4], op=ALU.mult)
            nc.vector.tensor_tensor(out=sc5[:, 1:2], in0=stat[:, 0:1],
                                    in1=sc5[:, 0:1], op=ALU.mult)
            nc.vector.tensor_tensor(out=sc5[:, 1:2], in0=g5[:, o5, 1:2],
                                    in1=sc5[:, 1:2], op=ALU.subtract)
            for qs in range(N // 512):
                sl = slice(512 * qs, 512 * (qs + 1))
                ot = st3.tile([P, 512], f32, tag="sq5o")
                nc.scalar.activation(ot[:], out5v[:, o5, sl], AF.Prelu,
                                     bias=sc5[:, 1:2], scale=sc5[:, 0:1],
                                     alpha=alpha_col[:, 0:1])
                nc.sync.dma_start(outT[P * o5:P * (o5 + 1), sl], ot[:])


def _host_prep(inputs):
    smp = np.asarray(inputs["sampled"], np.float32)
    ctr = np.asarray(inputs["center"], np.float32)
    act = np.asarray(inputs["action"], np.float32)
    w_in = np.asarray(inputs["w_in"], np.float32)
    b_in = np.asarray(inputs["b_in"], np.float32)
    ws = [np.asarray(inputs[f"w{i}"], np.float32) for i in (1, 2, 3, 4)]
    gws = [np.asarray(inputs[f"g{i}w"], np.float32) for i in (1, 2, 3, 4)]
    gbs = [np.asarray(inputs[f"g{i}b"], np.float32) for i in (1, 2, 3, 4)]
    w5 = np.asarray(inputs["w5"], np.float32)
    b5 = np.asarray(inputs["b5"], np.float32)
    g5w = np.asarray(inputs["g5w"], np.float32)
    g5b = np.asarray(inputs["g5b"], np.float32)

    shared = {}
    shared["wina"] = np.concatenate([w_in.T, b_in[None, :]], 0).astype(BF16)

    in_maps = []
    for core in range(8):
        b, h = core // 2, core % 2
        m = dict(shared)
        for i, (C, OH) in enumerate(BLOCKS):
            w = ws[i]
            hs = slice(h * OH, (h + 1) * OH)
            wl, wr = w[hs, :C], w[hs, C:]
            m[f"wstk{i+1}"] = np.concatenate([wl.T, (wr - wl).T], 1).astype(BF16)
            g = np.zeros((1, 2048), np.float32)
            g[0, :OH] = gws[i][hs]
            g[0, 1024:1024 + OH] = gbs[i][hs]
            m[f"gnp{i+1}"] = g
        h5 = slice(h * DDH, (h + 1) * DDH)
        w5a = np.zeros((19 * P, DDH), np.float32)
        w5a[:2304] = w5[h5].T
        w5a[2304] = b5[h5]
        m["w5a"] = w5a.astype(BF16)
        m["gn5t"] = np.stack([g5w[h5], g5b[h5]], 1).astype(np.float32)

        x = np.concatenate([smp[b].T, np.repeat(act[b][:, None], N, 1),
                            np.ones((1, N), np.float32)], 0)
        coor = ctr[b].T
        sq = (coor * coor).sum(0).astype(np.float32)
        m["xaug"] = x.astype(BF16)
        knl = np.concatenate([coor, np.ones((1, N), np.float32)], 0)
        m["knnl"] = knl[:, h * (N // 2):(h + 1) * (N // 2)].copy()
        m["knnr"] = np.concatenate([2.0 * coor, -sq[None, :]], 0)
        in_maps.append(m)
    return in_maps


def kernel(**inputs):
    for i in (1, 2, 3, 4):
        assert np.all(np.asarray(inputs[f"g{i}w"]) >= 0), \
            "kernel assumes non-negative GN weights (max/LeakyReLU commute)"
    if "nc" not in _cache:
        _cache["nc"] = _build_nc()
    nc = _cache["nc"]
    in_maps = _host_prep(inputs)
    res = run_bass_kernel_spmd(nc, in_maps, core_ids=list(range(8)))
    out = np.empty((B, N, DD), np.float32)
    for b in range(B):
        out[b, :, 0:DDH] = res.results[2 * b]["outT"].T
        out[b, :, DDH:DD] = res.results[2 * b + 1]["outT"].T
    return out


def _get_fast(in_maps):
    if "fast" in _cache:
        return _cache["fast"]
    import jax
    from jax.sharding import Mesh, PartitionSpec, NamedSharding
    from jax.experimental.shard_map import shard_map
    from concourse import bass2jax
    import concourse.mybir as mb
    nc = _cache["nc"]
    bass2jax.install_neuronx_cc_hook()
    partition_name = nc.partition_id_tensor.name if nc.partition_id_tensor else None
    in_names, out_names, out_avals, zero_outs = [], [], [], []
    for alloc in nc.m.functions[0].allocations:
        if not isinstance(alloc, mb.MemoryLocationSet):
            continue
        name = alloc.memorylocations[0].name
        if alloc.kind == "ExternalInput":
            if name != partition_name:
                in_names.append(name)
        elif alloc.kind == "ExternalOutput":
            out_names.append(name)
            shape = tuple(alloc.tensor_shape)
            dtype = mb.dt.np(alloc.dtype)
            out_avals.append(jax.core.ShapedArray(shape, dtype))
            zero_outs.append(np.zeros(shape, dtype))
    n_params = len(in_names)
    all_in = in_names + out_names + ([partition_name] if partition_name else [])

    def _body(*args):
        operands = list(args)
        if partition_name is not None:
            operands.append(bass2jax.partition_id_tensor())
        outs = bass2jax._bass_exec_p.bind(
            *operands, out_avals=tuple(out_avals), in_names=tuple(all_in),
            out_names=tuple(out_names), lowering_input_output_aliases=(),
            sim_require_finite=True, sim_require_nnan=True, nc=nc)
        return tuple(outs)

    devices = jax.devices()[:8]
    mesh = Mesh(np.asarray(devices), ("core",))
    fn = jax.jit(shard_map(_body, mesh=mesh,
                           in_specs=(PartitionSpec("core"),) * (n_params + len(out_names)),
                           out_specs=(PartitionSpec("core"),) * len(out_names),
                           check_rep=False), keep_unused=True)
    sharding = NamedSharding(mesh, PartitionSpec("core"))
    _cache["fast"] = (fn, in_names, out_names, zero_outs, sharding)
    return _cache["fast"]


def timed_run(inputs, reps=10):
    import time as _t
    import jax
    if "nc" not in _cache:
        _cache["nc"] = _build_nc()
    in_maps = _host_prep(inputs)
    fn, in_names, out_names, zero_outs, sharding = _get_fast(in_maps)
    concat_in = [np.concatenate([np.asarray(m[nm]) for m in in_maps], 0)
                 for nm in in_names]
    concat_zeros = [np.zeros((8 * z.shape[0], *z.shape[1:]), z.dtype)
                    for z in zero_outs]
    dev = [jax.device_put(a, sharding) for a in concat_in + concat_zeros]
    r = fn(*dev); jax.block_until_ready(r)
    times = []
    for _ in range(reps):
        t0 = _t.perf_counter()
        r = fn(*dev)
        jax.block_until_ready(r)
        times.append(_t.perf_counter() - t0)
    oi = out_names.index("outT")
    arr = np.asarray(r[oi]).reshape(8, DDH, N)
    out = np.empty((B, N, DD), np.float32)
    for b in range(B):
        out[b, :, 0:DDH] = arr[2 * b].T
        out[b, :, DDH:DD] = arr[2 * b + 1].T
    return times, out


# revision 15
# speedup vs baseline: 16.7897x; 5.0779x over previous
"""Trainium2 Bass kernel for nn_CGCNNDynamics (Point-BERT DGCNN dynamics head).

V2: pair-split (2 cores per batch element, channel halves at GN group
boundaries) + instruction-count-oriented restructure:
 - one combined [hT | sT] matmul pass per point chunk
 - one 4-neighbor indirect gather per query chunk (multi-index offset AP)
 - edge values e = g + s formed explicitly; GN stats = sum(e), sum(e^2)
   accumulated across the whole block in two [1, OH] PSUM rows via
   ones-stationary matmuls (2 per query chunk)
 - channel-major staging + XBAR dma_start_transpose (per-slice 128x128)
   replaces per-chunk PE transposes; one wide Prelu per output-channel chunk
 - merged DMAs (4-chunk htbl writes, 1 DMA per block for agin/fnext)
 - pair AllGather of bf16 channel halves between blocks
"""
import sys, os
sys.path.insert(0, "/opt/trn_rl_repo")
KSTOP = os.environ.get("KSTOP", "")
import contextlib
import numpy as np
import ml_dtypes

import concourse.bass as bass
import concourse.bacc as bacc
import concourse.mybir as mybir
import concourse.tile as tile
from concourse.tile import add_dep_helper
from concourse.bass_utils import run_bass_kernel_spmd

BF16 = ml_dtypes.bfloat16
P = 128
B, N, TD, AD, DD = 4, 2048, 256, 8, 512
DDH = DD // 2
CIN = TD + AD          # 264
K = 4
EPS = 1e-5
ALPHA = 0.2
NCH = N // P           # 16
BLOCKS = [(128, 128), (256, 256), (512, 256), (512, 512)]   # (C_full, O_half)
PAIRS = [[0, 1], [2, 3], [4, 5], [6, 7]]

_cache = {}


def _runs_for_chunk(OH, cc):
    """Global channel chunk [cc*128,(cc+1)*128) -> [(r, row0, p0, ln)] in the
    per-range AllGather outputs (rows: rank-half h first, then local)."""
    H2 = OH // 2
    out = []
    c0 = cc * P
    while c0 < (cc + 1) * P:
        h, local = c0 // OH, c0 % OH
        r = local // H2
        row = h * H2 + (local % H2)
        ln = min(H2 - (local % H2), (cc + 1) * P - c0)
        out.append((r, row, c0 - cc * P, ln))
        c0 += ln
    return out


def _build_nc():
    nc = bacc.Bacc("TRN2", target_bir_lowering=False, debug=False,
                   enable_asserts=False, num_devices=8)
    f32, bf16, u32 = mybir.dt.float32, mybir.dt.bfloat16, mybir.dt.uint32

    xaug = nc.dram_tensor("xaug", [CIN + 1, N], bf16, kind="ExternalInput").ap()
    wina = nc.dram_tensor("wina", [CIN + 1, P], bf16, kind="ExternalInput").ap()
    knnl = nc.dram_tensor("knnl", [4, N // 2], f32, kind="ExternalInput").ap()
    knnr = nc.dram_tensor("knnr", [4, N], f32, kind="ExternalInput").ap()
    wstk = [nc.dram_tensor(f"wstk{i+1}", [c, 2 * o], bf16, kind="ExternalInput").ap()
            for i, (c, o) in enumerate(BLOCKS)]
    gnp = [nc.dram_tensor(f"gnp{i+1}", [1, 2048], f32, kind="ExternalInput").ap()
           for i in range(4)]
    w5a = nc.dram_tensor("w5a", [19 * P, DDH], bf16, kind="ExternalInput").ap()
    gn5t = nc.dram_tensor("gn5t", [DDH, 2], f32, kind="ExternalInput").ap()
    outT = nc.dram_tensor("outT", [DDH, N], f32, kind="ExternalOutput").ap()

    htbl = [nc.dram_tensor(f"htbl{i+1}", [N, o], bf16, kind="Internal").ap()
            for i, (_, o) in enumerate(BLOCKS)]
    agin = [nc.dram_tensor(f"agin{i+1}", [o, N], bf16, kind="Internal").ap()
            for i, (_, o) in enumerate(BLOCKS)]
    agout = [[nc.dram_tensor(f"agx{i+1}_{r}", [o, N], bf16, kind="Internal").ap()
              for r in range(2)] for i, (_, o) in enumerate(BLOCKS)]
    idxin = nc.dram_tensor("idxin", [P, 64], mybir.dt.uint32, kind="Internal").ap()
    idxout = nc.dram_tensor("idxout", [2 * P, 64], mybir.dt.uint32,
                            kind="Internal").ap()

    with tile.TileContext(nc) as tc:
        _emit(nc, tc, xaug, wina, knnl, knnr, wstk, gnp, w5a, gn5t,
              outT, htbl, agin, agout, idxin, idxout)
    nc.compile()
    return nc


def _emit(nc, tc, xaug, wina, knnl, knnr, wstk, gnp, w5a, gn5t,
          outT, htbl, agin, agout, idxin, idxout):
    f32, bf16, u32 = mybir.dt.float32, mybir.dt.bfloat16, mybir.dt.uint32
    AX, ALU, AF = mybir.AxisListType, mybir.AluOpType, mybir.ActivationFunctionType
    from concourse.masks import make_identity

    ctx = contextlib.ExitStack()
    with ctx:
        fpool = ctx.enter_context(tc.tile_pool(name="fpool", bufs=2))
        wpool = ctx.enter_context(tc.tile_pool(name="wpool", bufs=1))
        gpool = ctx.enter_context(tc.tile_pool(name="gpool", bufs=3))
        ftrp = ctx.enter_context(tc.tile_pool(name="ftrp", bufs=2))
        epool = ctx.enter_context(tc.tile_pool(name="epool", bufs=3))
        fmaxp = ctx.enter_context(tc.tile_pool(name="fmaxp", bufs=1))
        stallp = ctx.enter_context(tc.tile_pool(name="stallp", bufs=1))
        dstwp = ctx.enter_context(tc.tile_pool(name="dstwp", bufs=1))
        hstp = ctx.enter_context(tc.tile_pool(name="hstp", bufs=1))
        st1 = ctx.enter_context(tc.tile_pool(name="st1", bufs=1))
        st3 = ctx.enter_context(tc.tile_pool(name="st3", bufs=2))
        smalls = ctx.enter_context(tc.tile_pool(name="smalls", bufs=1))
        ps_mm = ctx.enter_context(tc.tile_pool(name="ps_mm", bufs=4, space="PSUM"))
        ps_st = ctx.enter_context(tc.tile_pool(name="ps_st", bufs=1, space="PSUM"))
        ps_tr = ctx.enter_context(tc.tile_pool(name="ps_tr", bufs=1, space="PSUM"))

        # ---- constants ----
        idt = smalls.tile([P, P], f32, tag="idt")
        make_identity(nc, idt[:])
        ones_col = smalls.tile([P, 1], f32, tag="ones_col")
        nc.vector.memset(ones_col[:], 1.0)
        ones_colb = smalls.tile([P, 1], bf16, tag="ones_colb")
        nc.vector.memset(ones_colb[:], 1.0)
        ones_row = smalls.tile([1, P], f32, tag="ones_row")
        nc.vector.memset(ones_row[:], 1.0)
        ones_rhs = smalls.tile([P, 512], bf16, tag="ones_rhs")
        nc.vector.memset(ones_rhs[:], 1.0)
        alpha_col = smalls.tile([P, 1], f32, tag="alpha_col")
        nc.vector.memset(alpha_col[:], ALPHA)
        eps_b = smalls.tile([1, 1], f32, tag="eps_b")
        nc.vector.memset(eps_b[:], EPS)
        eps_col = smalls.tile([P, 1], f32, tag="eps_col")
        nc.vector.memset(eps_col[:], EPS)
        idx_all = smalls.tile([P, NCH, 8], u32, tag="idx")
        top8v = smalls.tile([P, 8], f32, tag="top8v")
        wia = smalls.tile([P, 3, P], bf16, tag="wia")
        g5 = smalls.tile([P, 2, 2], f32, tag="g5")
        acc = smalls.tile([P, 2, 8], f32, tag="acc")
        acc2 = smalls.tile([P, 2, 8], f32, tag="acc2")

        def _bail():
            z = st1.tile([P, 512], f32, tag="sq5", name="bailz")
            nc.vector.memset(z[:], 0.0)
            for o5_ in range(DDH // P):
                for qs_ in range(N // 512):
                    nc.sync.dma_start(
                        outT[P * o5_:P * (o5_ + 1), 512 * qs_:512 * (qs_ + 1)], z[:])
        # ============ Phase A: conv_in -> f0 (chan-part bf16) ============
        xg = wpool.tile([P, 3, N], bf16, tag="w")
        xgv = xg[:]
        nc.sync.dma_start(xgv[:, 0, :], xaug[0:P, :])
        nc.sync.dma_start(xgv[:, 1, :], xaug[P:2 * P, :])
        nc.sync.dma_start(xgv[0:9, 2, :], xaug[2 * P:CIN + 1, :])
        nc.sync.dma_start(wia[:, 0, :], wina[0:P, :])
        nc.sync.dma_start(wia[:, 1, :], wina[P:2 * P, :])
        nc.sync.dma_start(wia[0:9, 2, :], wina[2 * P:CIN + 1, :])

        f0 = fpool.tile([P, 4, N], bf16, tag="f", name="f0")
        for qs in range(N // 512):
            pt = ps_mm.tile([P, 512], f32, tag="mm")
            sl = slice(512 * qs, 512 * (qs + 1))
            nc.tensor.matmul(out=pt[:], lhsT=wia[:, 0, :], rhs=xgv[:, 0, sl],
                             start=True, stop=False)
            nc.tensor.matmul(out=pt[:], lhsT=wia[:, 1, :], rhs=xgv[:, 1, sl],
                             start=False, stop=False)
            nc.tensor.matmul(out=pt[:], lhsT=wia[0:9, 2, :], rhs=xgv[0:9, 2, sl],
                             start=False, stop=True)
            nc.scalar.copy(f0[:, 0, sl], pt[:])

        if KSTOP == "a":
            _bail()
            return
        # ============ Phase B: KNN top-4 indices ============
        kl = smalls.tile([4, N // 2], f32, tag="kl")
        nc.sync.dma_start(kl[:], knnl)
        kr = smalls.tile([4, N], f32, tag="kr")
        nc.sync.dma_start(kr[:], knnr)
        for qc in range(NCH // 2):
            dsb = fmaxp.tile([P, 4, NCH, P], bf16, tag="fmax", name=f"dv{qc}")
            dview = dsb[:].rearrange("p a b c -> p (a b c)").bitcast(f32)[:, 0:2048]
            for js in range(N // 512):
                pt = ps_mm.tile([P, 512], f32, tag="mm")
                nc.tensor.matmul(out=pt[:], lhsT=kl[:, P * qc:P * (qc + 1)],
                                 rhs=kr[:, 512 * js:512 * (js + 1)],
                                 start=True, stop=True)
                nc.scalar.copy(dview[:, 512 * js:512 * (js + 1)], pt[:])
            nc.vector.max(out=top8v[:], in_=dview)
            nc.vector.max_index(out=idx_all[:, qc, :], in_max=top8v[:],
                                in_values=dview)
        # exchange halves: own idx -> slot h of idxout
        iw = nc.sync.dma_start(idxin, idx_all[:, 0:NCH // 2, :].rearrange(
            "p c o -> p (c o)"))
        if os.environ.get("NOCC", ""):
            icc = nc.gpsimd.dma_start(idxout[0:P, :], idxin)
            ic2 = nc.gpsimd.dma_start(idxout[P:2 * P, :], idxin)
            add_dep_helper(ic2.ins, iw.ins, reason="idx ag after idxin")
        else:
            icc = nc.gpsimd.collective_compute(
                "AllGather", ALU.bypass, replica_groups=PAIRS,
                ins=[idxin], outs=[idxout])
        add_dep_helper(icc.ins, iw.ins, reason="idx ag after idxin")
        il1 = nc.sync.dma_start(idx_all[:, 0:NCH // 2, :].rearrange(
            "p c o -> p (c o)"), idxout[0:P, :])
        il2 = nc.sync.dma_start(idx_all[:, NCH // 2:NCH, :].rearrange(
            "p c o -> p (c o)"), idxout[P:2 * P, :])
        add_dep_helper(il1.ins, icc.ins, reason="idx load after ag")
        add_dep_helper(il2.ins, icc.ins, reason="idx load after ag")

        if KSTOP == "b":
            _bail()
            return
        # ============ Edge blocks ============
        fprev = f0
        prevCC = 1
        ag_cc = [None] * 4
        for bi, (C, OH) in enumerate(BLOCKS):
            CC = C // P
            OC = OH // P
            grp = OH // 2
            # one DMA for the whole weight stack [C, 2*OH] -> [P, CC, 2*OH]
            wk = wpool.tile([P, 8, 512], bf16, tag="w")
            wv = wk[:].rearrange("p c o -> p (c o)")[:, 0:CC * 2 * OH].rearrange(
                "p (c o) -> p c o", o=2 * OH)
            nc.scalar.dma_start(
                wv[:], wstk[bi].rearrange("(c p) o -> p c o", p=P))

            # ---- C.1: combined [hT | sT] pass ----
            # st_all: sT stash bf16 [P, NCH, OH]; htbl written 4 chunks/DMA
            st_all = stallp.tile([P, NCH, 512], bf16, tag="stall")
            h_w = []
            ngrp = 2 * OH // 512 if 2 * OH > 512 else 1
            for hc in range(NCH // 4):
                hstage = hstp.tile([P, 4, 512], bf16, tag="hst")
                for sub in range(4):
                    nchunk = 4 * hc + sub
                    fsl = slice(P * nchunk, P * (nchunk + 1))
                    if ngrp == 1:
                        pt = ps_mm.tile([P, 512], f32, tag="mm")
                        for cc in range(CC):
                            nc.tensor.matmul(
                                out=pt[:, 0:2 * OH],
                                lhsT=fprev[:, cc, fsl],
                                rhs=wv[:, cc, :],
                                start=(cc == 0), stop=(cc == CC - 1))
                        nc.scalar.copy(hstage[:, sub, 0:OH], pt[:, 0:OH])
                        nc.scalar.copy(st_all[:, nchunk, 0:OH], pt[:, OH:2 * OH])
                    else:
                        pa = ps_mm.tile([P, 512], f32, tag="mm")
                        pb = ps_mm.tile([P, 512], f32, tag="mm")
                        for cc in range(CC):
                            nc.tensor.matmul(
                                out=pa[:], lhsT=fprev[:, cc, fsl],
                                rhs=wv[:, cc, 0:OH],
                                start=(cc == 0), stop=(cc == CC - 1))
                            nc.tensor.matmul(
                                out=pb[:], lhsT=fprev[:, cc, fsl],
                                rhs=wv[:, cc, OH:2 * OH],
                                start=(cc == 0), stop=(cc == CC - 1))
                        nc.scalar.copy(hstage[:, sub, 0:OH], pa[:])
                        nc.scalar.copy(st_all[:, nchunk, 0:OH], pb[:])
                wi = nc.sync.dma_start(
                    htbl[bi][4 * P * hc:4 * P * (hc + 1), :].rearrange(
                        "(s p) o -> p s o", p=P),
                    hstage[:, :, 0:OH])
                h_w.append(wi)

            if KSTOP == f"c1_{bi+1}":
                _bail()
                return
            # ---- C.2: gather -> e -> max + stats ----
            ste = ps_st.tile([1, 512], f32, tag="ste", name=f"ste{bi}")
            stq = ps_st.tile([1, 512], f32, tag="stq", name=f"stq{bi}")
            # fmax2: channel-chunk-major staging [P, OC, NCH, 128]
            fmax2 = fmaxp.tile([P, 4, NCH, P], bf16, tag="fmax")
            for qc in range(NCH):
                g = gpool.tile([P, K, 512], bf16, tag="g")
                for k in range(K):
                    gi = nc.gpsimd.indirect_dma_start(
                        out=g[:, k, 0:OH], out_offset=None, in_=htbl[bi],
                        in_offset=bass.IndirectOffsetOnAxis(
                            ap=idx_all[:, qc, k:k + 1], axis=0))
                    for wi in h_w:
                        add_dep_helper(gi.ins, wi.ins, reason="gather after htbl")

                e = epool.tile([P, K, 512], bf16, tag="e")
                stb = st_all[:, qc, 0:OH].unsqueeze(1).broadcast_to([P, K, OH])
                nc.vector.tensor_tensor(out=e[:, 0:K, 0:OH], in0=g[:, :, 0:OH],
                                        in1=stb, op=ALU.add)
                # max over K: two pair-merged steps
                m2 = epool.tile([P, 2, 512], bf16, tag="m2")
                nc.vector.tensor_tensor(out=m2[:, :, 0:OH], in0=e[:, 0:2, 0:OH],
                                        in1=e[:, 2:4, 0:OH], op=ALU.max)
                nc.vector.tensor_tensor(
                    out=fmax2[:, 0:OC, qc, :],
                    in0=m2[:, 0, 0:OH].rearrange("p (c o) -> p c o", o=P),
                    in1=m2[:, 1, 0:OH].rearrange("p (c o) -> p c o", o=P),
                    op=ALU.max)
                # e sums: contiguous pair-merged adds on DVE
                s2 = epool.tile([P, 2, 512], bf16, tag="s2")
                nc.vector.tensor_tensor(out=s2[:, :, 0:OH], in0=e[:, 0:2, 0:OH],
                                        in1=e[:, 2:4, 0:OH], op=ALU.add)
                esum = epool.tile([P, 512], bf16, tag="esum")
                nc.vector.tensor_tensor(out=esum[:, 0:OH], in0=s2[:, 0, 0:OH],
                                        in1=s2[:, 1, 0:OH], op=ALU.add)
                # e^2 sums
                sqe = epool.tile([P, K, 512], bf16, tag="sqe")
                nc.scalar.activation(sqe[:, 0:K, 0:OH], e[:, 0:K, 0:OH], AF.Square)
                q2 = epool.tile([P, 2, 512], bf16, tag="q2")
                nc.vector.tensor_tensor(out=q2[:, :, 0:OH], in0=sqe[:, 0:2, 0:OH],
                                        in1=sqe[:, 2:4, 0:OH], op=ALU.add)
                qsum = epool.tile([P, 512], bf16, tag="qsum")
                nc.vector.tensor_tensor(out=qsum[:, 0:OH], in0=q2[:, 0, 0:OH],
                                        in1=q2[:, 1, 0:OH], op=ALU.add)
                # stats partition-reduction: accumulate in PSUM rows
                nc.tensor.matmul(out=ste[:, 0:OH], lhsT=ones_colb[:],
                                 rhs=esum[:, 0:OH], start=(qc == 0),
                                 stop=(qc == NCH - 1), skip_group_check=True)
                nc.tensor.matmul(out=stq[:, 0:OH], lhsT=ones_colb[:],
                                 rhs=qsum[:, 0:OH], start=(qc == 0),
                                 stop=(qc == NCH - 1), skip_group_check=True)

            if KSTOP == f"c2_{bi+1}":
                _bail()
                return
            # ---- C.3: group stats -> sc/bb rows on partition 0 ----
            gp = st1.tile([1, 2048], f32, tag="gp")
            nc.scalar.dma_start(gp[:], gnp[bi])
            combo = st1.tile([1, 2048], f32, tag="combo")  # [sc | bb] cols
            cnt = float(grp * N * K)
            gsum = st1.tile([1, 8], f32, tag="gsum")  # [mu*2 | e2m*2 | sd*2 | r*2]
            nc.vector.tensor_reduce(
                out=gsum[:, 0:2],
                in_=ste[:, 0:OH].rearrange("p (g c) -> p g c", g=2),
                axis=AX.X, op=ALU.add)
            nc.vector.tensor_reduce(
                out=gsum[:, 2:4],
                in_=stq[:, 0:OH].rearrange("p (g c) -> p g c", g=2),
                axis=AX.X, op=ALU.add)
            nc.vector.tensor_scalar_mul(gsum[:, 0:4], gsum[:, 0:4], 1.0 / cnt)
            musq = st1.tile([1, 2], f32, tag="musq")
            nc.vector.tensor_tensor(out=musq[:], in0=gsum[:, 0:2],
                                    in1=gsum[:, 0:2], op=ALU.mult)
            nc.vector.tensor_tensor(out=gsum[:, 2:4], in0=gsum[:, 2:4],
                                    in1=musq[:], op=ALU.subtract)
            nc.scalar.activation(gsum[:, 4:6], gsum[:, 2:4], AF.Sqrt,
                                 bias=eps_b[:, 0:1])
            nc.vector.reciprocal(gsum[:, 6:8], gsum[:, 4:6])
            rview = gsum[:, 6:8].unsqueeze(-1).broadcast_to([1, 2, grp])
            muv = gsum[:, 0:2].unsqueeze(-1).broadcast_to([1, 2, grp])
            scv = combo[:, 0:OH].rearrange("p (g c) -> p g c", g=2)
            bbv = combo[:, 1024:1024 + OH].rearrange("p (g c) -> p g c", g=2)
            gwv = gp[:, 0:OH].rearrange("p (g c) -> p g c", g=2)
            gbv = gp[:, 1024:1024 + OH].rearrange("p (g c) -> p g c", g=2)
            nc.vector.tensor_tensor(out=scv, in0=gwv, in1=rview, op=ALU.mult)
            nc.vector.tensor_tensor(out=bbv, in0=muv, in1=scv, op=ALU.mult)
            nc.vector.tensor_tensor(out=bbv, in0=gbv, in1=bbv, op=ALU.subtract)

            if KSTOP == f"c3_{bi+1}":
                _bail()
                return
            # ---- C.4: XBAR transpose per oc + wide Prelu -> agin DRAM ----
            dstw = dstwp.tile([P, 4, N], bf16, tag="dstw")
            for oc in range(OC):
                cps = ps_tr.tile([P, 2], f32, tag="tp")
                nc.tensor.transpose(out=cps[:, 0:1],
                                    in_=combo[:, P * oc:P * (oc + 1)],
                                    identity=idt[0:1, 0:1])
                nc.tensor.transpose(out=cps[:, 1:2],
                                    in_=combo[:, 1024 + P * oc:1024 + P * (oc + 1)],
                                    identity=idt[0:1, 0:1])
                col = st1.tile([P, 2], f32, tag="col")
                nc.scalar.copy(col[:], cps[:, 0:2])
                ftr = ftrp.tile([P, NCH, P], bf16, tag="ftr")
                nc.sync.dma_start_transpose(
                    ftr[:], fmax2[:, oc, :, :].rearrange("p s o -> p (s o)"))
                nc.scalar.activation(
                    dstw[:, oc, :], ftr[:].rearrange("p s o -> p (s o)"),
                    AF.Prelu, bias=col[:, 1:2], scale=col[:, 0:1],
                    alpha=alpha_col[:, 0:1])
            # own-half rows [r*H2,(r+1)*H2) per range; write + gather each
            H2 = OH // 2
            ccs = []
            for r in range(2):
                rs = slice(r * H2, (r + 1) * H2)
                if H2 >= P:
                    wi = nc.scalar.dma_start(
                        agin[bi][rs, :].rearrange("(c p) n -> p c n", p=P),
                        dstw[:, r * H2 // P:(r + 1) * H2 // P, :])
                else:
                    wi = nc.scalar.dma_start(agin[bi][rs, :],
                                             dstw[r * H2:(r + 1) * H2, 0, :])
                if os.environ.get("NOCC", ""):
                    c1 = nc.gpsimd.dma_start(agout[bi][r][0:H2, :],
                                             agin[bi][rs, :])
                    c2 = nc.gpsimd.dma_start(agout[bi][r][H2:OH, :],
                                             agin[bi][rs, :])
                    add_dep_helper(c1.ins, wi.ins, reason="ag after agin")
                    add_dep_helper(c2.ins, wi.ins, reason="ag after agin")
                    ccs.append(c2)
                else:
                    c1 = nc.gpsimd.collective_compute(
                        "AllGather", ALU.bypass, replica_groups=PAIRS,
                        ins=[agin[bi][rs, :]], outs=[agout[bi][r]])
                    add_dep_helper(c1.ins, wi.ins, reason="ag after agin")
                    ccs.append(c1)
            ag_cc[bi] = ccs

            # ---- load full f_next from the range outputs ----
            if bi < 3:
                CCn = 2 * OH // P
                fnext = fpool.tile([P, 4, N], bf16, tag="f", name=f"f{bi+1}")
                for cc in range(CCn):
                    for (r, row0, p0, ln) in _runs_for_chunk(OH, cc):
                        li = nc.scalar.dma_start(
                            fnext[p0:p0 + ln, cc, :],
                            agout[bi][r][row0:row0 + ln, :])
                        add_dep_helper(li.ins, ccs[r].ins,
                                       reason="fnext after allgather")
            else:
                fnext = None
            fprev = fnext
            if KSTOP == f"blk{bi+1}":
                _bail()
                return

        # ============ Final conv5 + GN5 + LeakyReLU (own DDH half) ==========
        w5t = wpool.tile([P, 19, DDH], bf16, tag="w")
        w5v = w5t[:]
        nc.sync.dma_start(w5v[:], w5a.rearrange("(c p) o -> p c o", p=P))
        nc.sync.dma_start(g5[:], gn5t.rearrange("(o p) s -> p o s", p=P))

        cmap = []
        for bi, (_, o) in enumerate(BLOCKS):
            for r in range(2 * o // P):
                cmap.append((bi, r))
        assert len(cmap) == 18

        out5 = fmaxp.tile([P, 4, NCH, P], bf16, tag="fmax")  # reuse slot bytes
        out5v = out5[:].rearrange("p a b c -> p (a b c)").bitcast(
            mybir.dt.float32).rearrange("p (c n) -> p c n", n=N)  # [P, 2, N] f32
        for half in range(2):
            hsl = slice(1024 * half, 1024 * (half + 1))
            pts = [[ps_mm.tile([P, 512], f32, tag="mm", name=f"pt5_{half}_{q}_{j}")
                    for j in range(2)] for q in range(2)]
            for cc in range(18):
                bi_, r_ = cmap[cc]
                OHb = BLOCKS[bi_][1]
                fct = st3.tile([P, 1024], bf16, tag="fcl")
                for (r, row0, p0, ln) in _runs_for_chunk(OHb, r_):
                    li = nc.scalar.dma_start(
                        fct[p0:p0 + ln, :],
                        agout[bi_][r][row0:row0 + ln, hsl])
                    add_dep_helper(li.ins, ag_cc[bi_][r].ins,
                                   reason="fc after allgather")
                for o5 in range(2):
                    for q in range(2):
                        nc.tensor.matmul(out=pts[q][o5][:],
                                         lhsT=w5v[:, cc, P * o5:P * (o5 + 1)],
                                         rhs=fct[:, 512 * q:512 * (q + 1)],
                                         start=(cc == 0), stop=False)
            for o5 in range(2):
                for q in range(2):
                    qs = 2 * half + q
                    sl = slice(512 * qs, 512 * (qs + 1))
                    nc.tensor.matmul(out=pts[q][o5][:],
                                     lhsT=w5v[:, 18, P * o5:P * (o5 + 1)],
                                     rhs=ones_rhs[:], start=False, stop=True)
                    nc.scalar.activation(out5v[:, o5, sl], pts[q][o5][:],
                                         AF.Identity, accum_out=acc[:, o5, qs:qs + 1])
                    sq5 = st1.tile([P, 512], f32, tag="sq5")
                    nc.scalar.activation(sq5[:], pts[q][o5][:], AF.Square,
                                         accum_out=acc2[:, o5, qs:qs + 1])

        for o5 in range(2):
            s1 = st1.tile([P, 2], f32, tag="s5")
            nc.vector.tensor_reduce(out=s1[:, 0:1], in_=acc[:, o5, 0:4],
                                    axis=AX.X, op=ALU.add)
            nc.vector.tensor_reduce(out=s1[:, 1:2], in_=acc2[:, o5, 0:4],
                                    axis=AX.X, op=ALU.add)
            tot = ps_st.tile([1, 2], f32, tag="ste")
            nc.tensor.matmul(out=tot[:], lhsT=ones_col[:], rhs=s1[:],
                             start=True, stop=True)
            stot = st1.tile([1, 2], f32, tag="stot")
            nc.scalar.copy(stot[:], tot[:])
            bc = ps_st.tile([P, 2], f32, tag="stq")
            nc.tensor.matmul(out=bc[:], lhsT=ones_row[:], rhs=stot[:],
                             start=True, stop=True)
            stat = st1.tile([P, 4], f32, tag="stat")   # [mu, var, sd, r]
            nc.scalar.copy(stat[:, 0:2], bc[:])
            nc.vector.tensor_scalar_mul(stat[:, 0:2], stat[:, 0:2], 1.0 / (P * N))
            mu2 = st1.tile([P, 1], f32, tag="mu2")
            nc.vector.tensor_tensor(out=mu2[:], in0=stat[:, 0:1], in1=stat[:, 0:1],
                                    op=ALU.mult)
            nc.vector.tensor_tensor(out=stat[:, 1:2], in0=stat[:, 1:2], in1=mu2[:],
                                    op=ALU.subtract)
            nc.scalar.activation(stat[:, 2:3], stat[:, 1:2], AF.Sqrt,
                                 bias=eps_col[:, 0:1])
            nc.vector.reciprocal(stat[:, 3:4], stat[:, 2:3])
            sc5 = st1.tile([P, 2], f32, tag="sc5")
            nc.vector.tensor_tensor(out=sc5[:, 0:1], in0=g5[:, o5, 0:1],
                                    in1=stat[:, 3:4], op=ALU.mult)
            nc.vector.tensor_tensor(out=sc5[:, 1:2], in0=stat[:, 0:1],
                                    in1=sc5[:, 0:1], op=ALU.mult)
            nc.vector.tensor_tensor(out=sc5[:, 1:2], in0=g5[:, o5, 1:2],
                                    in1=sc5[:, 1:2], op=ALU.subtract)
            for qs in range(N // 512):
                sl = slice(512 * qs, 512 * (qs + 1))
                ot = st3.tile([P, 512], f32, tag="sq5o")
                nc.scalar.activation(ot[:], out5v[:, o5, sl], AF.Prelu,
                                     bias=sc5[:, 1:2], scale=sc5[:, 0:1],
                                     alpha=alpha_col[:, 0:1])
                nc.sync.dma_start(outT[P * o5:P * (o5 + 1), sl], ot[:])


def _host_prep(inputs):
    smp = np.asarray(inputs["sampled"], np.float32)
    ctr = np.asarray(inputs["center"], np.float32)
    act = np.asarray(inputs["action"], np.float32)
    w_in = np.asarray(inputs["w_in"], np.float32)
    b_in = np.asarray(inputs["b_in"], np.float32)
    ws = [np.asarray(inputs[f"w{i}"], np.float32) for i in (1, 2, 3, 4)]
    gws = [np.asarray(inputs[f"g{i}w"], np.float32) for i in (1, 2, 3, 4)]
    gbs = [np.asarray(inputs[f"g{i}b"], np.float32) for i in (1, 2, 3, 4)]
    w5 = np.asarray(inputs["w5"], np.float32)
    b5 = np.asarray(inputs["b5"], np.float32)
    g5w = np.asarray(inputs["g5w"], np.float32)
    g5b = np.asarray(inputs["g5b"], np.float32)

    shared = {}
    shared["wina"] = np.concatenate([w_in.T, b_in[None, :]], 0).astype(BF16)

    in_maps = []
    for core in range(8):
        b, h = core // 2, core % 2
        m = dict(shared)
        for i, (C, OH) in enumerate(BLOCKS):
            w = ws[i]
            hs = slice(h * OH, (h + 1) * OH)
            wl, wr = w[hs, :C], w[hs, C:]
            m[f"wstk{i+1}"] = np.concatenate([wl.T, (wr - wl).T], 1).astype(BF16)
            g = np.zeros((1, 2048), np.float32)
            g[0, :OH] = gws[i][hs]
            g[0, 1024:1024 + OH] = gbs[i][hs]
            m[f"gnp{i+1}"] = g
        h5 = slice(h * DDH, (h + 1) * DDH)
        w5a = np.zeros((19 * P, DDH), np.float32)
        w5a[:2304] = w5[h5].T
        w5a[2304] = b5[h5]
        m["w5a"] = w5a.astype(BF16)
        m["gn5t"] = np.stack([g5w[h5], g5b[h5]], 1).astype(np.float32)

        x = np.concatenate([smp[b].T, np.repeat(act[b][:, None], N, 1),
                            np.ones((1, N), np.float32)], 0)
        coor = ctr[b].T
        sq = (coor * coor).sum(0).astype(np.float32)
        m["xaug"] = x.astype(BF16)
        knl = np.concatenate([coor, np.ones((1, N), np.float32)], 0)
        m["knnl"] = knl[:, h * (N // 2):(h + 1) * (N // 2)].copy()
        m["knnr"] = np.concatenate([2.0 * coor, -sq[None, :]], 0)
        in_maps.append(m)
    return in_maps


def kernel(**inputs):
    for i in (1, 2, 3, 4):
        assert np.all(np.asarray(inputs[f"g{i}w"]) >= 0), \
            "kernel assumes non-negative GN weights (max/LeakyReLU commute)"
    if "nc" not in _cache:
        _cache["nc"] = _build_nc()
    nc = _cache["nc"]
    in_maps = _host_prep(inputs)
    res = run_bass_kernel_spmd(nc, in_maps, core_ids=list(range(8)))
    out = np.empty((B, N, DD), np.float32)
    for b in range(B):
        out[b, :, 0:DDH] = res.results[2 * b]["outT"].T
        out[b, :, DDH:DD] = res.results[2 * b + 1]["outT"].T
    return out


def _get_fast(in_maps):
    if "fast" in _cache:
        return _cache["fast"]
    import jax
    from jax.sharding import Mesh, PartitionSpec, NamedSharding
    from jax.experimental.shard_map import shard_map
    from concourse import bass2jax
    import concourse.mybir as mb
    nc = _cache["nc"]
    bass2jax.install_neuronx_cc_hook()
    partition_name = nc.partition_id_tensor.name if nc.partition_id_tensor else None
    in_names, out_names, out_avals, zero_outs = [], [], [], []
    for alloc in nc.m.functions[0].allocations:
        if not isinstance(alloc, mb.MemoryLocationSet):
            continue
        name = alloc.memorylocations[0].name
        if alloc.kind == "ExternalInput":
            if name != partition_name:
                in_names.append(name)
        elif alloc.kind == "ExternalOutput":
            out_names.append(name)
            shape = tuple(alloc.tensor_shape)
            dtype = mb.dt.np(alloc.dtype)
            out_avals.append(jax.core.ShapedArray(shape, dtype))
            zero_outs.append(np.zeros(shape, dtype))
    n_params = len(in_names)
    all_in = in_names + out_names + ([partition_name] if partition_name else [])

    def _body(*args):
        operands = list(args)
        if partition_name is not None:
            operands.append(bass2jax.partition_id_tensor())
        outs = bass2jax._bass_exec_p.bind(
            *operands, out_avals=tuple(out_avals), in_names=tuple(all_in),
            out_names=tuple(out_names), lowering_input_output_aliases=(),
            sim_require_finite=True, sim_require_nnan=True, nc=nc)
        return tuple(outs)

    devices = jax.devices()[:8]
    mesh = Mesh(np.asarray(devices), ("core",))
    fn = jax.jit(shard_map(_body, mesh=mesh,
                           in_specs=(PartitionSpec("core"),) * (n_params + len(out_names)),
                           out_specs=(PartitionSpec("core"),) * len(out_names),
                           check_rep=False), keep_unused=True)
    sharding = NamedSharding(mesh, PartitionSpec("core"))
    _cache["fast"] = (fn, in_names, out_names, zero_outs, sharding)
    return _cache["fast"]


def timed_run(inputs, reps=10):
    import time as _t
    import jax
    if "nc" not in _cache:
        _cache["nc"] = _build_nc()
    in_maps = _host_prep(inputs)
    fn, in_names, out_names, zero_outs, sharding = _get_fast(in_maps)
    concat_in = [np.concatenate([np.asarray(m[nm]) for m in in_maps], 0)
                 for nm in in_names]
    concat_zeros = [np.zeros((8 * z.shape[0], *z.shape[1:]), z.dtype)
                    for z in zero_outs]
    dev = [jax.device_put(a, sharding) for a in concat_in + concat_zeros]
    r = fn(*dev); jax.block_until_ready(r)
    times = []
    for _ in range(reps):
        t0 = _t.perf_counter()
        r = fn(*dev)
        jax.block_until_ready(r)
        times.append(_t.perf_counter() - t0)
    oi = out_names.index("outT")
    arr = np.asarray(r[oi]).reshape(8, DDH, N)
    out = np.empty((B, N, DD), np.float32)
    for b in range(B):
        out[b, :, 0:DDH] = arr[2 * b].T
        out[b, :, DDH:DD] = arr[2 * b + 1].T
    return times, out
